# revision 38
# baseline (speedup 1.0000x reference)
"""Trainium2 Bass kernel for nn_DN1 (developmental-network step).

Computation (see problem reference):
  inpt = concat(x, y_response, z)                    # (13312,)
  response = neurons @ inpt                          # (9216,)  <- memory-bound bulk
  top-k selection on z part (k=8+1) and y part (k=16+1), normalized scores
  scattered into final_rsp; the 24 selected rows get an age-weighted
  running-average update + renorm.

Distribution: neurons is row-sharded across 8 NeuronCores (1152 rows each).
Launch A computes the local matvec partials per shard with a fused
multiply+accumulate (scalar_tensor_tensor with accum_out) on the Vector
engine, in natural [row, D] layout (no transpose anywhere). The tiny top-k
over 9216 values + score normalization runs on host from the gathered
response ("topk per-shard then globally reduced" degenerates to a host
reduce at this size). Launch B updates the 24 selected rows on device (PE
for the age-weighted row sums, DVE for the axpy + renorm); the host scatters
the returned rows into the full-size outputs.

Kernels are written in raw Bass (explicit engine blocks + semaphores):
the walrus build in this container only supports a single sync-wait per
instruction, which rules out the Tile scheduler's fused-wait output, so all
waits are standalone sequencer instructions.
"""
import numpy as np

import concourse.bass as bass
import concourse.mybir as mybir
from concourse.bass_utils import run_bass_kernel_spmd

F32 = mybir.dt.float32

NCORES = 8
X_SIZE, NUM_NEURONS, Z_SIZE = 4096, 8192, 1024
N_TOTAL = NUM_NEURONS + Z_SIZE          # 9216
D = X_SIZE + NUM_NEURONS + Z_SIZE       # 13312
RPC = N_TOTAL // NCORES                 # 1152 rows per core
NTILES = RPC // 128                     # 9
TOPK, ZTOPK = 16, 8
TIE_RAND = 0.5

_progs: dict = {}
last_perf: list = []  # (label, exec_time_ns or None) for the last kernel() call


def _build_matvec(nchunk=2, wbufs=4, ibc_gpsimd=False, ibc_pe=False):
    """Per-core: resp[p, t] = dot(wshard[t*128+p, :], inpt).

    nchunk: D split per row-tile (load granularity = [128, D/nchunk]);
    wbufs: load slot double-buffering depth;
    ibc_gpsimd: replicate inpt across partitions with a GpSimd
      partition_broadcast (custom op — does not compile on this walrus);
    ibc_pe: replicate inpt via ones[1,128] matmuls through PSUM + DVE
      copies, avoiding the 6.8MB stride-0 broadcast DMA. Needs the "inpt"
      input as [2, D] with row 1 = ones.
    """
    dchunk = D // nchunk
    nmm = D // 512                        # 26 psum-bank blocks
    assert not (ibc_pe and ibc_gpsimd)
    assert not ibc_pe or dchunk % 512 == 0
    # segment list (tile, lo, hi): last tile split finer so the final stt
    # (which can only start once the last DMA lands) is short
    segs = []
    for t in range(NTILES):
        if t == NTILES - 1:
            q = dchunk // 4
            for lo in range(0, D, q):
                segs.append((t, lo, lo + q))
        else:
            for c in range(nchunk):
                segs.append((t, c * dchunk, (c + 1) * dchunk))
    ntot = len(segs)
    tile_end = {}  # tile -> 1-based tick of its last stt
    for i, (t, lo, hi) in enumerate(segs):
        tile_end[t] = i + 1
    tile_cols = {}  # tile -> (first acc col, n cols)
    for i, (t, lo, hi) in enumerate(segs):
        if t not in tile_cols:
            tile_cols[t] = [i, 0]
        tile_cols[t][1] += 1
    nc = bass.Bass()
    w = nc.dram_tensor("wshard", [RPC, D], F32, kind="ExternalInput")
    inpt = nc.dram_tensor("inpt", [D + 128] if ibc_pe else [D], F32,
                          kind="ExternalInput")
    resp = nc.dram_tensor("resp", [128, NTILES], F32, kind="ExternalOutput")
    with (
        nc.sbuf_tensor([128, D], F32) as ibc,
        nc.sbuf_tensor([1, 128], F32) as ones_sb,
        nc.sbuf_tensor([128, wbufs, dchunk], F32) as wt,
        nc.sbuf_tensor([128, ntot], F32) as acc,
        nc.sbuf_tensor([128, NTILES], F32) as resp_sb,
        nc.psum_tensor([128, 8, 512], F32) as ps,
        nc.semaphore("s_gp") as s_gp,
        nc.semaphore("s_pe") as s_pe,
        nc.semaphore("s_cp") as s_cp,
        nc.semaphore("s_dve") as s_dve,
        nc.semaphore("s_red") as s_red,
        nc.semaphore("s_out") as s_out,
        nc.Block() as block,
    ):
        s_ibc = [nc.alloc_semaphore(f"s_ibc{c}") for c in range(nchunk)]
        s_wt = [nc.alloc_semaphore(f"s_w{k}") for k in range(wbufs)]

        @block.sync
        def _(sync):
            if ibc_pe:
                # stage inpt in ibc row 0 (the broadcast copies rewrite it
                # with identical values), ones tail into its own tile
                sync.dma_start(ibc[0:1, :], inpt[None, 0:D]).then_inc(
                    s_ibc[0], 16
                )
                sync.dma_start(ones_sb[:], inpt[None, D:D + 128]).then_inc(
                    s_ibc[0], 16
                )
            elif ibc_gpsimd:
                sync.dma_start(ibc[0:1, :], inpt[None, :]).then_inc(s_ibc[0], 16)
            else:
                for c in range(nchunk):
                    sync.dma_start(
                        ibc[:, c * dchunk:(c + 1) * dchunk],
                        inpt[c * dchunk:(c + 1) * dchunk].partition_broadcast(128),
                    ).then_inc(s_ibc[c], 16)
            for i, (t, lo, hi) in enumerate(segs):
                k = i % wbufs
                if i >= wbufs:
                    # slot reuse: wait until stt_{i-wbufs} consumed the slot
                    sync.wait_ge(s_dve, i - wbufs + 1)
                sync.dma_start(
                    wt[:, k, 0:hi - lo],
                    w[t * 128:(t + 1) * 128, lo:hi],
                ).then_inc(s_wt[k], 16)
            sync.wait_ge(s_red, NTILES)
            sync.dma_start(resp[:], resp_sb[:]).then_inc(s_out, 16)
            sync.wait_ge(s_out, 16)

        if ibc_gpsimd:
            @block.gpsimd
            def _(gpsimd):
                from concourse import library_config
                gpsimd.load_library(library_config.proxy)
                gpsimd.wait_ge(s_ibc[0], 16)
                nc.gpsimd.partition_broadcast(ibc[:], ibc[0:1, :]).then_inc(s_gp, 1)

        if ibc_pe:
            @block.tensor
            def _(tensor):
                tensor.wait_ge(s_ibc[0], 32)
                for m in range(nmm):
                    b = m % 8
                    if m >= 8:
                        tensor.wait_ge(s_cp, m - 7)
                    nc.tensor.matmul(
                        ps[:, b, :],
                        ones_sb[:],
                        ibc[0:1, m * 512:(m + 1) * 512],
                        start=True,
                        stop=True,
                    ).then_inc(s_pe, 1)

        @block.vector
        def _(vector):
            if ibc_pe:
                for m in range(nmm):
                    vector.wait_ge(s_pe, m + 1)
                    nc.vector.tensor_copy(
                        ibc[:, m * 512:(m + 1) * 512], ps[:, m % 8, :]
                    ).then_inc(s_cp, 1)
            cp_waited = 0
            seen_chunk = set()
            for i, (t, lo, hi) in enumerate(segs):
                k = i % wbufs
                c = lo // dchunk
                if c not in seen_chunk:
                    seen_chunk.add(c)
                    if ibc_pe:
                        need = -(-hi // 512)  # psum blocks covering this seg
                        if need > cp_waited:
                            vector.wait_ge(s_cp, need)
                            cp_waited = need
                    elif ibc_gpsimd:
                        if not seen_chunk - {c}:
                            vector.wait_ge(s_gp, 1)
                    else:
                        vector.wait_ge(s_ibc[c], 16)
                vector.wait_ge(s_wt[k], 16 * (i // wbufs + 1))
                nc.vector.scalar_tensor_tensor(
                    out=wt[:, k, 0:hi - lo],
                    in0=wt[:, k, 0:hi - lo],
                    scalar=1.0,
                    in1=ibc[:, lo:hi],
                    op0=mybir.AluOpType.mult,
                    op1=mybir.AluOpType.mult,
                    accum_out=acc[:, i:i + 1],
                ).then_inc(s_dve, 1)
                if i + 1 == tile_end[t]:
                    # fold this tile's partials into resp while later tiles
                    # are still loading (keeps the reduce off the tail)
                    col0, ncol = tile_cols[t]
                    vector.wait_ge(s_dve, tile_end[t])
                    nc.vector.reduce_sum(
                        resp_sb[:, t:t + 1],
                        acc[:, col0:col0 + ncol],
                        axis=mybir.AxisListType.X,
                    ).then_inc(s_red, 1)
    return nc


def _build_update(nk):
    """newrows[i] = normalize(s_{branch(i)} + inpt / a_i), with
    s_b = sum_j coef_j rows_j over rows j in branch b.

    Inputs are packed in one tensor to keep DMA count low:
      rowscat [nk, D + nk + 1]: cols [0:D) selected rows, [D:D+nk) the
      branch-masked coefficient matrix (lhsT layout), col D+nk = 1/ages.
    """
    mchunk = 512
    nmm = D // mchunk                    # 26
    pbufs = 8                            # PSUM banks used
    gsize = 4                            # PSUM banks consumed per DVE op
    groups = []                          # (first chunk, n chunks) per DVE group
    m0 = 0
    while m0 < nmm:
        gw = min(gsize, nmm - m0)
        assert (m0 % pbufs) + gw <= pbufs
        groups.append((m0, gw))
        m0 += gw
    ngroups = len(groups)
    # DVE tick numbers: group stts 1..ngroups, reduce, recip, tsmul
    t_reduce = ngroups + 1
    t_recip = ngroups + 2
    t_tsmul = ngroups + 3

    nc = bass.Bass()
    rowscat = nc.dram_tensor("rowscat", [nk, D + nk + 1], F32, kind="ExternalInput")
    inpt = nc.dram_tensor("inpt", [D], F32, kind="ExternalInput")
    newrows = nc.dram_tensor("newrows", [nk, D], F32, kind="ExternalOutput")
    with (
        nc.sbuf_tensor([nk, D + nk + 1], F32) as rows_sb,
        nc.sbuf_tensor([nk, D], F32) as new_sb,
        nc.sbuf_tensor([nk, D], F32) as ibc,
        nc.sbuf_tensor([nk, gsize * mchunk], F32) as sq_trash,
        nc.sbuf_tensor([nk, ngroups], F32) as sscols,
        nc.sbuf_tensor([nk, 1], F32) as ss,
        nc.sbuf_tensor([nk, 1], F32) as rn,
        nc.psum_tensor([nk, pbufs, mchunk], F32) as s_ps,
        nc.semaphore("s_row") as s_row,
        nc.semaphore("s_ibb") as s_ibb,
        nc.semaphore("s_pe") as s_pe,
        nc.semaphore("s_dve") as s_dve,
        nc.semaphore("s_act") as s_act,
        nc.semaphore("s_out") as s_out,
        nc.Block() as block,
    ):
        @block.sync
        def _(sync):
            sync.dma_start(rows_sb[:], rowscat[:]).then_inc(s_row, 16)
            sync.dma_start(ibc[:], inpt[:].partition_broadcast(nk)).then_inc(
                s_ibb, 16
            )
            sync.wait_ge(s_dve, t_tsmul)
            sync.dma_start(newrows[:], new_sb[:]).then_inc(s_out, 16)
            sync.wait_ge(s_out, 16)

        @block.tensor
        def _(tensor):
            tensor.wait_ge(s_row, 16)
            for m in range(nmm):
                b = m % pbufs
                if m >= pbufs:
                    # psum bank reuse: wait until the group that consumed
                    # chunk m-pbufs has run
                    gidx = next(
                        gi for gi, (g0, gw) in enumerate(groups)
                        if g0 <= m - pbufs < g0 + gw
                    )
                    tensor.wait_ge(s_dve, gidx + 1)
                nc.tensor.matmul(
                    s_ps[:, b, :],
                    rows_sb[:, D:D + nk],
                    rows_sb[:, m * mchunk:(m + 1) * mchunk],
                    start=True,
                    stop=True,
                ).then_inc(s_pe, 1)

        @block.vector
        def _(vector):
            vector.wait_ge(s_ibb, 16)
            vector.wait_ge(s_row, 16)
            for gi, (g0, gw) in enumerate(groups):
                b0 = g0 % pbufs
                vector.wait_ge(s_pe, g0 + gw)
                sl = slice(g0 * mchunk, (g0 + gw) * mchunk)
                nc.vector.scalar_tensor_tensor(
                    out=new_sb[:, sl],
                    in0=ibc[:, sl],
                    scalar=rows_sb[:, D + nk:D + nk + 1],
                    in1=s_ps[:, b0:b0 + gw, :],
                    op0=mybir.AluOpType.mult,
                    op1=mybir.AluOpType.add,
                ).then_inc(s_dve, 1)
            # norm tail: squares were accumulated per-group on ACT
            vector.wait_ge(s_act, ngroups)
            nc.vector.reduce_sum(
                ss[:], sscols[:], axis=mybir.AxisListType.X
            ).then_inc(s_dve, 1)
            vector.wait_ge(s_act, ngroups + 1)
            nc.vector.reciprocal(rn[:], rn[:]).then_inc(s_dve, 1)
            vector.wait_ge(s_dve, t_recip)
            nc.vector.tensor_scalar_mul(new_sb[:], new_sb[:], rn[:]).then_inc(
                s_dve, 1
            )

        @block.scalar
        def _(scalar):
            for gi, (g0, gw) in enumerate(groups):
                scalar.wait_ge(s_dve, gi + 1)
                if gi > 0:
                    # same-engine WAW on the trash buffer needs a sem
                    scalar.wait_ge(s_act, gi)
                sl = slice(g0 * mchunk, (g0 + gw) * mchunk)
                nc.scalar.activation(
                    sq_trash[:, 0:gw * mchunk],
                    new_sb[:, sl],
                    mybir.ActivationFunctionType.Square,
                    accum_out=sscols[:, gi:gi + 1],
                ).then_inc(s_act, 1)
            scalar.wait_ge(s_dve, t_reduce)
            nc.scalar.activation(
                rn[:], ss[:], mybir.ActivationFunctionType.Sqrt
            ).then_inc(s_act, 1)
    return nc


def _build_update2(nk, kz):
    """128-partition-layout row update: partition p holds segment
    [p*SEG:(p+1)*SEG) of every selected row, so every DMA and vector op runs
    at full port width (v1 ran everything on nk=24 partitions -> 3/16 SDMA
    engines and 24/128 DVE lanes).

    Inputs:
      rows24 [nk, D]    selected rows (z rows first)
      inpt   [D]
      extras [128, 128 + 2*nk]: cols [0:128) ones (matmul reducer),
             [128:128+nk) coef=(a-1)/a, [128+nk:128+2nk) 1/a
    Output:
      newrows [nk, D]

    Per-branch s = sum_i coef_i rows_i becomes a free-dim strided reduce of
    the coef-scaled rows; the cross-partition sum-of-squares reduce+broadcast
    is one ones[128,128] matmul.
    """
    seg = D // 128                       # 104
    ky = nk - kz
    nc = bass.Bass()
    rows24 = nc.dram_tensor("rows24", [nk, D], F32, kind="ExternalInput")
    inpt = nc.dram_tensor("inpt", [D], F32, kind="ExternalInput")
    extras = nc.dram_tensor("extras", [128, 128 + 2 * nk], F32, kind="ExternalInput")
    newrows = nc.dram_tensor("newrows", [nk, D], F32, kind="ExternalOutput")
    with (
        nc.sbuf_tensor([128, nk * seg], F32) as R,
        nc.sbuf_tensor([128, nk * seg], F32) as W,
        nc.sbuf_tensor([128, seg], F32) as ibc,
        nc.sbuf_tensor([128, seg], F32) as s_z,
        nc.sbuf_tensor([128, seg], F32) as s_y,
        nc.sbuf_tensor([128, 128 + 2 * nk], F32) as ex,
        nc.sbuf_tensor([128, nk], F32) as ssqp,
        nc.sbuf_tensor([128, nk], F32) as rn,
        nc.psum_tensor([128, nk], F32) as ps_ssq,
        nc.semaphore("s_r") as s_r,
        nc.semaphore("s_i") as s_i,
        nc.semaphore("s_e") as s_e,
        nc.semaphore("s_pe") as s_pe,
        nc.semaphore("s_dve") as s_dve,
        nc.semaphore("s_act") as s_act,
        nc.semaphore("s_out") as s_out,
        nc.Block() as block,
    ):
        Rv = R[:].rearrange("p (i j) -> p i j", i=nk)      # [128, nk, seg]
        Wv = W[:].rearrange("p (i j) -> p i j", i=nk)
        coef_bc = ex[:, 128:128 + nk][:, :, None].broadcast_to((128, nk, seg))
        inva_bc = ex[:, 128 + nk:128 + 2 * nk][:, :, None].broadcast_to(
            (128, nk, seg)
        )
        ibc_bc = ibc[:][:, None, :].broadcast_to((128, nk, seg))
        rn_bc = rn[:][:, :, None].broadcast_to((128, nk, seg))

        @block.sync
        def _(sync):
            sync.dma_start(
                Rv, rows24[:].rearrange("i (p j) -> p i j", p=128)
            ).then_inc(s_r, 16)
            sync.dma_start(
                ibc[:], inpt[:].rearrange("(p j) -> p j", p=128)
            ).then_inc(s_i, 16)
            sync.dma_start(ex[:], extras[:]).then_inc(s_e, 16)
            sync.wait_ge(s_dve, 9)
            sync.dma_start(
                newrows[:].rearrange("i (p j) -> p i j", p=128), Rv
            ).then_inc(s_out, 16)
            sync.wait_ge(s_out, 16)

        @block.vector
        def _(vector):
            # 1: W = inpt x (1/a) — needs only the small early inputs, so it
            # runs while the big rows DMA is still streaming
            vector.wait_ge(s_i, 16)
            vector.wait_ge(s_e, 16)
            nc.vector.tensor_mul(Wv, ibc_bc, inva_bc).then_inc(s_dve, 1)
            vector.wait_ge(s_r, 16)
            # 2: coef-scale the rows in place
            nc.vector.tensor_mul(Rv, Rv, coef_bc).then_inc(s_dve, 1)
            vector.wait_ge(s_dve, 2)
            # 3,4: per-branch s = strided reduce over the row axis
            nc.vector.reduce_sum(
                s_z[:], Rv[:, 0:kz, :].transpose([0, 2, 1]),
                axis=mybir.AxisListType.X,
            ).then_inc(s_dve, 1)
            nc.vector.reduce_sum(
                s_y[:], Rv[:, kz:nk, :].transpose([0, 2, 1]),
                axis=mybir.AxisListType.X,
            ).then_inc(s_dve, 1)
            vector.wait_ge(s_dve, 4)
            # 5,6: new = W + s_branch (overwrites R)
            nc.vector.tensor_add(
                Rv[:, 0:kz, :], Wv[:, 0:kz, :],
                s_z[:][:, None, :].broadcast_to((128, kz, seg)),
            ).then_inc(s_dve, 1)
            nc.vector.tensor_add(
                Rv[:, kz:nk, :], Wv[:, kz:nk, :],
                s_y[:][:, None, :].broadcast_to((128, ky, seg)),
            ).then_inc(s_dve, 1)
            # 7: per-partition sumsq segments (squares by ACT into W)
            vector.wait_ge(s_act, 1)
            nc.vector.reduce_sum(
                ssqp[:], Wv, axis=mybir.AxisListType.X
            ).then_inc(s_dve, 1)
            # 8: 1/sqrt after PE reduce-broadcast + ACT sqrt
            vector.wait_ge(s_act, 2)
            nc.vector.reciprocal(rn[:], rn[:]).then_inc(s_dve, 1)
            vector.wait_ge(s_dve, 8)
            # 9: scale rows by 1/norm
            nc.vector.tensor_mul(Rv, Rv, rn_bc).then_inc(s_dve, 1)

        @block.tensor
        def _(tensor):
            tensor.wait_ge(s_dve, 7)
            # ones[128,128] @ ssqp: cross-partition sum AND broadcast in one op
            nc.tensor.matmul(
                ps_ssq[:], ex[:, 0:128], ssqp[:], start=True, stop=True
            ).then_inc(s_pe, 1)

        @block.scalar
        def _(scalar):
            scalar.wait_ge(s_dve, 6)
            nc.scalar.activation(
                Wv, Rv, mybir.ActivationFunctionType.Square
            ).then_inc(s_act, 1)
            scalar.wait_ge(s_pe, 1)
            nc.scalar.activation(
                rn[:], ps_ssq[:], mybir.ActivationFunctionType.Sqrt
            ).then_inc(s_act, 1)
    return nc


def _get_prog(key, builder):
    if key not in _progs:
        _progs[key] = builder()
    return _progs[key]


def kernel(x, z, neurons, ages, y_response, num_neurons_init):
    global last_perf
    last_perf = []
    x = np.ascontiguousarray(np.asarray(x, dtype=np.float32))
    z = np.ascontiguousarray(np.asarray(z, dtype=np.float32))
    neurons = np.ascontiguousarray(np.asarray(neurons, dtype=np.float32))
    ages = np.ascontiguousarray(np.asarray(ages, dtype=np.float32))
    y_response = np.ascontiguousarray(np.asarray(y_response, dtype=np.float32))
    nni = int(np.asarray(num_neurons_init))

    inpt = np.concatenate([x, y_response, z]).astype(np.float32)
    inpt_ones = np.concatenate([inpt, np.ones(128, np.float32)])

    # ---- launch A: distributed matvec (row-sharded) ----
    nc_a = _get_prog(
        "matvec", lambda: _build_matvec(nchunk=2, wbufs=5, ibc_pe=True)
    )
    in_maps = [
        {"wshard": neurons[c * RPC:(c + 1) * RPC], "inpt": inpt_ones}
        for c in range(NCORES)
    ]
    res_a = run_bass_kernel_spmd(nc_a, in_maps, core_ids=list(range(NCORES)))
    last_perf.append(("matvec", res_a.exec_time_ns))
    response = np.concatenate(
        [res_a.results[c]["resp"].T.ravel() for c in range(NCORES)]
    )

    # ---- host: global top-k reduce + normalized scores (tiny: 9216 values) ----
    ytk = 1 if nni <= TOPK else TOPK
    ztk = 1 if nni <= TOPK else ZTOPK

    zresp = response[NUM_NEURONS:]
    zord = np.argsort(-zresp, kind="stable")[:ztk + 1]
    zvals = zresp[zord]
    zsel = NUM_NEURONS + zord[:-1]
    zscore = (zvals[:-1] - zvals[-1]) / (zvals[0] - zvals[-1])

    yresp = response[:NUM_NEURONS]
    yord = np.argsort(-yresp, kind="stable")[:ytk + 1]
    yvals = yresp[yord]
    t = np.float32(1.0 if np.any(yvals[:-1] == yvals[-1]) else 0.0)
    denom = yvals[0] - yvals[-1] + np.float32(1e-9) * (t * np.float32(TIE_RAND))
    ysel = yord[:-1]
    yscore = (yvals[:-1] - yvals[-1]) / denom

    # ---- launch B: update + renorm the selected rows on device ----
    idx = np.concatenate([zsel, ysel])
    nk = len(idx)
    a_sel = ages[idx]
    coef = ((a_sel - np.float32(1.0)) / a_sel).astype(np.float32)
    inva = (np.float32(1.0) / a_sel).astype(np.float32)
    extras = np.ones((128, 128 + 2 * nk), np.float32)
    extras[:, 128:128 + nk] = coef[None, :]
    extras[:, 128 + nk:] = inva[None, :]
    rows = np.ascontiguousarray(neurons[idx])

    nc_b = _get_prog(("update2", nk, ztk), lambda: _build_update2(nk, ztk))
    res_b = run_bass_kernel_spmd(
        nc_b,
        [{"rows24": rows, "inpt": inpt, "extras": extras}],
        core_ids=[0],
    )
    last_perf.append(("update", res_b.exec_time_ns))
    newrows = res_b.results[0]["newrows"]

    # ---- host: unshard / assemble full-shape outputs ----
    final_rsp = np.zeros(N_TOTAL, np.float32)
    final_rsp[zsel] = zscore
    final_rsp[ysel] = yscore
    neurons_out = neurons.copy()
    neurons_out[idx] = newrows
    ages_out = ages.copy()
    ages_out[idx] += np.float32(1.0)

    return final_rsp[NUM_NEURONS:], final_rsp[:NUM_NEURONS], neurons_out, ages_out


# revision 40
# speedup vs baseline: 1.0059x; 1.0059x over previous
"""Trainium2 Bass kernel for nn_DN1 (developmental-network step).

Computation (see problem reference):
  inpt = concat(x, y_response, z)                    # (13312,)
  response = neurons @ inpt                          # (9216,)  <- memory-bound bulk
  top-k selection on z part (k=8+1) and y part (k=16+1), normalized scores
  scattered into final_rsp; the 24 selected rows get an age-weighted
  running-average update + renorm.

Distribution: neurons is row-sharded across 8 NeuronCores (1152 rows each).
Launch A computes the local matvec partials per shard with a fused
multiply+accumulate (scalar_tensor_tensor with accum_out) on the Vector
engine, in natural [row, D] layout (no transpose anywhere). The tiny top-k
over 9216 values + score normalization runs on host from the gathered
response ("topk per-shard then globally reduced" degenerates to a host
reduce at this size). Launch B updates the 24 selected rows on device (PE
for the age-weighted row sums, DVE for the axpy + renorm); the host scatters
the returned rows into the full-size outputs.

Kernels are written in raw Bass (explicit engine blocks + semaphores):
the walrus build in this container only supports a single sync-wait per
instruction, which rules out the Tile scheduler's fused-wait output, so all
waits are standalone sequencer instructions.
"""
import numpy as np

import concourse.bass as bass
import concourse.mybir as mybir
from concourse.bass_utils import run_bass_kernel_spmd

F32 = mybir.dt.float32

NCORES = 8
X_SIZE, NUM_NEURONS, Z_SIZE = 4096, 8192, 1024
N_TOTAL = NUM_NEURONS + Z_SIZE          # 9216
D = X_SIZE + NUM_NEURONS + Z_SIZE       # 13312
RPC = N_TOTAL // NCORES                 # 1152 rows per core
NTILES = RPC // 128                     # 9
TOPK, ZTOPK = 16, 8
TIE_RAND = 0.5

_progs: dict = {}
last_perf: list = []  # (label, exec_time_ns or None) for the last kernel() call


def _build_matvec(nchunk=2, wbufs=4, ibc_gpsimd=False, ibc_pe=False):
    """Per-core: resp[p, t] = dot(wshard[t*128+p, :], inpt).

    nchunk: D split per row-tile (load granularity = [128, D/nchunk]);
    wbufs: load slot double-buffering depth;
    ibc_gpsimd: replicate inpt across partitions with a GpSimd
      partition_broadcast (custom op — does not compile on this walrus);
    ibc_pe: replicate inpt via ones[1,128] matmuls through PSUM + DVE
      copies, avoiding the 6.8MB stride-0 broadcast DMA. Needs the "inpt"
      input as [2, D] with row 1 = ones.
    """
    dchunk = D // nchunk
    nmm = D // 512                        # 26 psum-bank blocks
    assert not (ibc_pe and ibc_gpsimd)
    assert not ibc_pe or dchunk % 512 == 0
    # segment list (tile, lo, hi): last tile split finer so the final stt
    # (which can only start once the last DMA lands) is short
    segs = []
    for t in range(NTILES):
        if t == NTILES - 1:
            q = dchunk // 4
            for lo in range(0, D, q):
                segs.append((t, lo, lo + q))
        else:
            for c in range(nchunk):
                segs.append((t, c * dchunk, (c + 1) * dchunk))
    ntot = len(segs)
    tile_end = {}  # tile -> 1-based tick of its last stt
    for i, (t, lo, hi) in enumerate(segs):
        tile_end[t] = i + 1
    tile_cols = {}  # tile -> (first acc col, n cols)
    for i, (t, lo, hi) in enumerate(segs):
        if t not in tile_cols:
            tile_cols[t] = [i, 0]
        tile_cols[t][1] += 1
    nc = bass.Bass()
    w = nc.dram_tensor("wshard", [RPC, D], F32, kind="ExternalInput")
    inpt = nc.dram_tensor("inpt", [D + 128] if ibc_pe else [D], F32,
                          kind="ExternalInput")
    resp = nc.dram_tensor("resp", [128, NTILES], F32, kind="ExternalOutput")
    with (
        nc.sbuf_tensor([128, D], F32) as ibc,
        nc.sbuf_tensor([1, 128], F32) as ones_sb,
        nc.sbuf_tensor([128, wbufs, dchunk], F32) as wt,
        nc.sbuf_tensor([128, ntot], F32) as acc,
        nc.sbuf_tensor([128, NTILES], F32) as resp_sb,
        nc.psum_tensor([128, 8, 512], F32) as ps,
        nc.semaphore("s_gp") as s_gp,
        nc.semaphore("s_pe") as s_pe,
        nc.semaphore("s_cp") as s_cp,
        nc.semaphore("s_dve") as s_dve,
        nc.semaphore("s_red") as s_red,
        nc.semaphore("s_out") as s_out,
        nc.Block() as block,
    ):
        s_ibc = [nc.alloc_semaphore(f"s_ibc{c}") for c in range(nchunk)]
        s_wt = [nc.alloc_semaphore(f"s_w{k}") for k in range(wbufs)]

        @block.sync
        def _(sync):
            if ibc_pe:
                # stage inpt in ibc row 0 (the broadcast copies rewrite it
                # with identical values), ones tail into its own tile
                sync.dma_start(ibc[0:1, :], inpt[None, 0:D]).then_inc(
                    s_ibc[0], 16
                )
                sync.dma_start(ones_sb[:], inpt[None, D:D + 128]).then_inc(
                    s_ibc[0], 16
                )
            elif ibc_gpsimd:
                sync.dma_start(ibc[0:1, :], inpt[None, :]).then_inc(s_ibc[0], 16)
            else:
                for c in range(nchunk):
                    sync.dma_start(
                        ibc[:, c * dchunk:(c + 1) * dchunk],
                        inpt[c * dchunk:(c + 1) * dchunk].partition_broadcast(128),
                    ).then_inc(s_ibc[c], 16)
            for i, (t, lo, hi) in enumerate(segs):
                k = i % wbufs
                if i >= wbufs:
                    # slot reuse: wait until stt_{i-wbufs} consumed the slot
                    sync.wait_ge(s_dve, i - wbufs + 1)
                sync.dma_start(
                    wt[:, k, 0:hi - lo],
                    w[t * 128:(t + 1) * 128, lo:hi],
                ).then_inc(s_wt[k], 16)
            sync.wait_ge(s_red, NTILES)
            sync.dma_start(resp[:], resp_sb[:]).then_inc(s_out, 16)
            sync.wait_ge(s_out, 16)

        if ibc_gpsimd:
            @block.gpsimd
            def _(gpsimd):
                from concourse import library_config
                gpsimd.load_library(library_config.proxy)
                gpsimd.wait_ge(s_ibc[0], 16)
                nc.gpsimd.partition_broadcast(ibc[:], ibc[0:1, :]).then_inc(s_gp, 1)

        if ibc_pe:
            @block.tensor
            def _(tensor):
                tensor.wait_ge(s_ibc[0], 32)
                for m in range(nmm):
                    b = m % 8
                    if m >= 8:
                        tensor.wait_ge(s_cp, m - 7)
                    nc.tensor.matmul(
                        ps[:, b, :],
                        ones_sb[:],
                        ibc[0:1, m * 512:(m + 1) * 512],
                        start=True,
                        stop=True,
                    ).then_inc(s_pe, 1)

        @block.vector
        def _(vector):
            if ibc_pe:
                for m in range(nmm):
                    vector.wait_ge(s_pe, m + 1)
                    nc.vector.tensor_copy(
                        ibc[:, m * 512:(m + 1) * 512], ps[:, m % 8, :]
                    ).then_inc(s_cp, 1)
            cp_waited = 0
            seen_chunk = set()
            for i, (t, lo, hi) in enumerate(segs):
                k = i % wbufs
                c = lo // dchunk
                if c not in seen_chunk:
                    seen_chunk.add(c)
                    if ibc_pe:
                        need = -(-hi // 512)  # psum blocks covering this seg
                        if need > cp_waited:
                            vector.wait_ge(s_cp, need)
                            cp_waited = need
                    elif ibc_gpsimd:
                        if not seen_chunk - {c}:
                            vector.wait_ge(s_gp, 1)
                    else:
                        vector.wait_ge(s_ibc[c], 16)
                vector.wait_ge(s_wt[k], 16 * (i // wbufs + 1))
                nc.vector.scalar_tensor_tensor(
                    out=wt[:, k, 0:hi - lo],
                    in0=wt[:, k, 0:hi - lo],
                    scalar=1.0,
                    in1=ibc[:, lo:hi],
                    op0=mybir.AluOpType.mult,
                    op1=mybir.AluOpType.mult,
                    accum_out=acc[:, i:i + 1],
                ).then_inc(s_dve, 1)
                if i + 1 == tile_end[t]:
                    # fold this tile's partials into resp while later tiles
                    # are still loading (keeps the reduce off the tail)
                    col0, ncol = tile_cols[t]
                    vector.wait_ge(s_dve, tile_end[t])
                    nc.vector.reduce_sum(
                        resp_sb[:, t:t + 1],
                        acc[:, col0:col0 + ncol],
                        axis=mybir.AxisListType.X,
                    ).then_inc(s_red, 1)
    return nc


def _build_update(nk):
    """newrows[i] = normalize(s_{branch(i)} + inpt / a_i), with
    s_b = sum_j coef_j rows_j over rows j in branch b.

    Inputs are packed in one tensor to keep DMA count low:
      rowscat [nk, D + nk + 1]: cols [0:D) selected rows, [D:D+nk) the
      branch-masked coefficient matrix (lhsT layout), col D+nk = 1/ages.
    """
    mchunk = 512
    nmm = D // mchunk                    # 26
    pbufs = 8                            # PSUM banks used
    gsize = 4                            # PSUM banks consumed per DVE op
    groups = []                          # (first chunk, n chunks) per DVE group
    m0 = 0
    while m0 < nmm:
        gw = min(gsize, nmm - m0)
        assert (m0 % pbufs) + gw <= pbufs
        groups.append((m0, gw))
        m0 += gw
    ngroups = len(groups)
    # DVE tick numbers: group stts 1..ngroups, reduce, recip, tsmul
    t_reduce = ngroups + 1
    t_recip = ngroups + 2
    t_tsmul = ngroups + 3

    nc = bass.Bass()
    rowscat = nc.dram_tensor("rowscat", [nk, D + nk + 1], F32, kind="ExternalInput")
    inpt = nc.dram_tensor("inpt", [D], F32, kind="ExternalInput")
    newrows = nc.dram_tensor("newrows", [nk, D], F32, kind="ExternalOutput")
    with (
        nc.sbuf_tensor([nk, D + nk + 1], F32) as rows_sb,
        nc.sbuf_tensor([nk, D], F32) as new_sb,
        nc.sbuf_tensor([nk, D], F32) as ibc,
        nc.sbuf_tensor([nk, gsize * mchunk], F32) as sq_trash,
        nc.sbuf_tensor([nk, ngroups], F32) as sscols,
        nc.sbuf_tensor([nk, 1], F32) as ss,
        nc.sbuf_tensor([nk, 1], F32) as rn,
        nc.psum_tensor([nk, pbufs, mchunk], F32) as s_ps,
        nc.semaphore("s_row") as s_row,
        nc.semaphore("s_ibb") as s_ibb,
        nc.semaphore("s_pe") as s_pe,
        nc.semaphore("s_dve") as s_dve,
        nc.semaphore("s_act") as s_act,
        nc.semaphore("s_out") as s_out,
        nc.Block() as block,
    ):
        @block.sync
        def _(sync):
            sync.dma_start(rows_sb[:], rowscat[:]).then_inc(s_row, 16)
            sync.dma_start(ibc[:], inpt[:].partition_broadcast(nk)).then_inc(
                s_ibb, 16
            )
            sync.wait_ge(s_dve, t_tsmul)
            sync.dma_start(newrows[:], new_sb[:]).then_inc(s_out, 16)
            sync.wait_ge(s_out, 16)

        @block.tensor
        def _(tensor):
            tensor.wait_ge(s_row, 16)
            for m in range(nmm):
                b = m % pbufs
                if m >= pbufs:
                    # psum bank reuse: wait until the group that consumed
                    # chunk m-pbufs has run
                    gidx = next(
                        gi for gi, (g0, gw) in enumerate(groups)
                        if g0 <= m - pbufs < g0 + gw
                    )
                    tensor.wait_ge(s_dve, gidx + 1)
                nc.tensor.matmul(
                    s_ps[:, b, :],
                    rows_sb[:, D:D + nk],
                    rows_sb[:, m * mchunk:(m + 1) * mchunk],
                    start=True,
                    stop=True,
                ).then_inc(s_pe, 1)

        @block.vector
        def _(vector):
            vector.wait_ge(s_ibb, 16)
            vector.wait_ge(s_row, 16)
            for gi, (g0, gw) in enumerate(groups):
                b0 = g0 % pbufs
                vector.wait_ge(s_pe, g0 + gw)
                sl = slice(g0 * mchunk, (g0 + gw) * mchunk)
                nc.vector.scalar_tensor_tensor(
                    out=new_sb[:, sl],
                    in0=ibc[:, sl],
                    scalar=rows_sb[:, D + nk:D + nk + 1],
                    in1=s_ps[:, b0:b0 + gw, :],
                    op0=mybir.AluOpType.mult,
                    op1=mybir.AluOpType.add,
                ).then_inc(s_dve, 1)
            # norm tail: squares were accumulated per-group on ACT
            vector.wait_ge(s_act, ngroups)
            nc.vector.reduce_sum(
                ss[:], sscols[:], axis=mybir.AxisListType.X
            ).then_inc(s_dve, 1)
            vector.wait_ge(s_act, ngroups + 1)
            nc.vector.reciprocal(rn[:], rn[:]).then_inc(s_dve, 1)
            vector.wait_ge(s_dve, t_recip)
            nc.vector.tensor_scalar_mul(new_sb[:], new_sb[:], rn[:]).then_inc(
                s_dve, 1
            )

        @block.scalar
        def _(scalar):
            for gi, (g0, gw) in enumerate(groups):
                scalar.wait_ge(s_dve, gi + 1)
                if gi > 0:
                    # same-engine WAW on the trash buffer needs a sem
                    scalar.wait_ge(s_act, gi)
                sl = slice(g0 * mchunk, (g0 + gw) * mchunk)
                nc.scalar.activation(
                    sq_trash[:, 0:gw * mchunk],
                    new_sb[:, sl],
                    mybir.ActivationFunctionType.Square,
                    accum_out=sscols[:, gi:gi + 1],
                ).then_inc(s_act, 1)
            scalar.wait_ge(s_dve, t_reduce)
            nc.scalar.activation(
                rn[:], ss[:], mybir.ActivationFunctionType.Sqrt
            ).then_inc(s_act, 1)
    return nc


def _build_update2(nk, kz):
    """128-partition-layout row update: partition p holds segment
    [p*SEG:(p+1)*SEG) of every selected row, so every DMA and vector op runs
    at full port width (v1 ran everything on nk=24 partitions -> 3/16 SDMA
    engines and 24/128 DVE lanes).

    Inputs:
      rows24 [nk, D]    selected rows (z rows first)
      inpt   [D]
      extras [128, 128 + 2*nk]: cols [0:128) ones (matmul reducer),
             [128:128+nk) coef=(a-1)/a, [128+nk:128+2nk) 1/a
    Output:
      newrows [nk, D]

    Per-branch s = sum_i coef_i rows_i becomes a free-dim strided reduce of
    the coef-scaled rows; the cross-partition sum-of-squares reduce+broadcast
    is one ones[128,128] matmul.
    """
    seg = D // 128                       # 104
    ky = nk - kz
    nc = bass.Bass()
    rows24 = nc.dram_tensor("rows24", [nk, D], F32, kind="ExternalInput")
    inpt = nc.dram_tensor("inpt", [D], F32, kind="ExternalInput")
    extras = nc.dram_tensor("extras", [128, 128 + 2 * nk], F32, kind="ExternalInput")
    newrows = nc.dram_tensor("newrows", [nk, D], F32, kind="ExternalOutput")
    with (
        nc.sbuf_tensor([128, nk * seg], F32) as R,
        nc.sbuf_tensor([128, nk * seg], F32) as W,
        nc.sbuf_tensor([128, seg], F32) as ibc,
        nc.sbuf_tensor([128, seg], F32) as s_z,
        nc.sbuf_tensor([128, seg], F32) as s_y,
        nc.sbuf_tensor([128, 128 + 2 * nk], F32) as ex,
        nc.sbuf_tensor([128, nk], F32) as ssqp,
        nc.sbuf_tensor([128, nk], F32) as rn,
        nc.psum_tensor([128, nk], F32) as ps_ssq,
        nc.semaphore("s_r") as s_r,
        nc.semaphore("s_i") as s_i,
        nc.semaphore("s_e") as s_e,
        nc.semaphore("s_pe") as s_pe,
        nc.semaphore("s_dve") as s_dve,
        nc.semaphore("s_act") as s_act,
        nc.semaphore("s_out") as s_out,
        nc.Block() as block,
    ):
        Rv = R[:].rearrange("p (i j) -> p i j", i=nk)      # [128, nk, seg]
        Wv = W[:].rearrange("p (i j) -> p i j", i=nk)
        coef_bc = ex[:, 128:128 + nk][:, :, None].broadcast_to((128, nk, seg))
        inva_bc = ex[:, 128 + nk:128 + 2 * nk][:, :, None].broadcast_to(
            (128, nk, seg)
        )
        ibc_bc = ibc[:][:, None, :].broadcast_to((128, nk, seg))
        rn_bc = rn[:][:, :, None].broadcast_to((128, nk, seg))

        @block.sync
        def _(sync):
            sync.dma_start(
                Rv, rows24[:].rearrange("i (p j) -> p i j", p=128)
            ).then_inc(s_r, 16)
            sync.dma_start(
                ibc[:], inpt[:].rearrange("(p j) -> p j", p=128)
            ).then_inc(s_i, 16)
            sync.dma_start(ex[:], extras[:]).then_inc(s_e, 16)
            # store in row-halves so the first DMA overlaps the second scale
            nro = newrows[:].rearrange("i (p j) -> p i j", p=128)
            sync.wait_ge(s_dve, 9)
            sync.dma_start(nro[:, 0:nk // 2, :], Rv[:, 0:nk // 2, :]).then_inc(
                s_out, 16
            )
            sync.wait_ge(s_dve, 10)
            sync.dma_start(nro[:, nk // 2:, :], Rv[:, nk // 2:, :]).then_inc(
                s_out, 16
            )
            sync.wait_ge(s_out, 32)

        @block.vector
        def _(vector):
            # 1: W = inpt x (1/a) — needs only the small early inputs, so it
            # runs while the big rows DMA is still streaming
            vector.wait_ge(s_i, 16)
            vector.wait_ge(s_e, 16)
            nc.vector.tensor_mul(Wv, ibc_bc, inva_bc).then_inc(s_dve, 1)
            vector.wait_ge(s_r, 16)
            # 2: coef-scale the rows in place
            nc.vector.tensor_mul(Rv, Rv, coef_bc).then_inc(s_dve, 1)
            vector.wait_ge(s_dve, 2)
            # 3,4: per-branch s = strided reduce over the row axis
            nc.vector.reduce_sum(
                s_z[:], Rv[:, 0:kz, :].transpose([0, 2, 1]),
                axis=mybir.AxisListType.X,
            ).then_inc(s_dve, 1)
            nc.vector.reduce_sum(
                s_y[:], Rv[:, kz:nk, :].transpose([0, 2, 1]),
                axis=mybir.AxisListType.X,
            ).then_inc(s_dve, 1)
            vector.wait_ge(s_dve, 4)
            # 5,6: new = W + s_branch (overwrites R)
            nc.vector.tensor_add(
                Rv[:, 0:kz, :], Wv[:, 0:kz, :],
                s_z[:][:, None, :].broadcast_to((128, kz, seg)),
            ).then_inc(s_dve, 1)
            nc.vector.tensor_add(
                Rv[:, kz:nk, :], Wv[:, kz:nk, :],
                s_y[:][:, None, :].broadcast_to((128, ky, seg)),
            ).then_inc(s_dve, 1)
            # 7: per-partition sumsq segments (squares by ACT into W)
            vector.wait_ge(s_act, 1)
            nc.vector.reduce_sum(
                ssqp[:], Wv, axis=mybir.AxisListType.X
            ).then_inc(s_dve, 1)
            # 8: 1/sqrt after PE reduce-broadcast + ACT sqrt
            vector.wait_ge(s_act, 2)
            nc.vector.reciprocal(rn[:], rn[:]).then_inc(s_dve, 1)
            vector.wait_ge(s_dve, 8)
            # 9,10: scale rows by 1/norm, in halves (overlaps the first store)
            h = nk // 2
            nc.vector.tensor_mul(
                Rv[:, 0:h, :], Rv[:, 0:h, :],
                rn[:, 0:h][:, :, None].broadcast_to((128, h, seg)),
            ).then_inc(s_dve, 1)
            nc.vector.tensor_mul(
                Rv[:, h:nk, :], Rv[:, h:nk, :],
                rn[:, h:nk][:, :, None].broadcast_to((128, nk - h, seg)),
            ).then_inc(s_dve, 1)

        @block.tensor
        def _(tensor):
            tensor.wait_ge(s_dve, 7)
            # ones[128,128] @ ssqp: cross-partition sum AND broadcast in one op
            nc.tensor.matmul(
                ps_ssq[:], ex[:, 0:128], ssqp[:], start=True, stop=True
            ).then_inc(s_pe, 1)

        @block.scalar
        def _(scalar):
            scalar.wait_ge(s_dve, 6)
            nc.scalar.activation(
                Wv, Rv, mybir.ActivationFunctionType.Square
            ).then_inc(s_act, 1)
            scalar.wait_ge(s_pe, 1)
            nc.scalar.activation(
                rn[:], ps_ssq[:], mybir.ActivationFunctionType.Sqrt
            ).then_inc(s_act, 1)
    return nc


def _get_prog(key, builder):
    if key not in _progs:
        _progs[key] = builder()
    return _progs[key]


def kernel(x, z, neurons, ages, y_response, num_neurons_init):
    global last_perf
    last_perf = []
    x = np.ascontiguousarray(np.asarray(x, dtype=np.float32))
    z = np.ascontiguousarray(np.asarray(z, dtype=np.float32))
    neurons = np.ascontiguousarray(np.asarray(neurons, dtype=np.float32))
    ages = np.ascontiguousarray(np.asarray(ages, dtype=np.float32))
    y_response = np.ascontiguousarray(np.asarray(y_response, dtype=np.float32))
    nni = int(np.asarray(num_neurons_init))

    inpt = np.concatenate([x, y_response, z]).astype(np.float32)
    inpt_ones = np.concatenate([inpt, np.ones(128, np.float32)])

    # ---- launch A: distributed matvec (row-sharded) ----
    nc_a = _get_prog(
        "matvec", lambda: _build_matvec(nchunk=2, wbufs=5, ibc_pe=True)
    )
    in_maps = [
        {"wshard": neurons[c * RPC:(c + 1) * RPC], "inpt": inpt_ones}
        for c in range(NCORES)
    ]
    res_a = run_bass_kernel_spmd(nc_a, in_maps, core_ids=list(range(NCORES)))
    last_perf.append(("matvec", res_a.exec_time_ns))
    response = np.concatenate(
        [res_a.results[c]["resp"].T.ravel() for c in range(NCORES)]
    )

    # ---- host: global top-k reduce + normalized scores (tiny: 9216 values) ----
    ytk = 1 if nni <= TOPK else TOPK
    ztk = 1 if nni <= TOPK else ZTOPK

    zresp = response[NUM_NEURONS:]
    zord = np.argsort(-zresp, kind="stable")[:ztk + 1]
    zvals = zresp[zord]
    zsel = NUM_NEURONS + zord[:-1]
    zscore = (zvals[:-1] - zvals[-1]) / (zvals[0] - zvals[-1])

    yresp = response[:NUM_NEURONS]
    yord = np.argsort(-yresp, kind="stable")[:ytk + 1]
    yvals = yresp[yord]
    t = np.float32(1.0 if np.any(yvals[:-1] == yvals[-1]) else 0.0)
    denom = yvals[0] - yvals[-1] + np.float32(1e-9) * (t * np.float32(TIE_RAND))
    ysel = yord[:-1]
    yscore = (yvals[:-1] - yvals[-1]) / denom

    # ---- launch B: update + renorm the selected rows on device ----
    idx = np.concatenate([zsel, ysel])
    nk = len(idx)
    a_sel = ages[idx]
    coef = ((a_sel - np.float32(1.0)) / a_sel).astype(np.float32)
    inva = (np.float32(1.0) / a_sel).astype(np.float32)
    extras = np.ones((128, 128 + 2 * nk), np.float32)
    extras[:, 128:128 + nk] = coef[None, :]
    extras[:, 128 + nk:] = inva[None, :]
    rows = np.ascontiguousarray(neurons[idx])

    nc_b = _get_prog(("update2", nk, ztk), lambda: _build_update2(nk, ztk))
    res_b = run_bass_kernel_spmd(
        nc_b,
        [{"rows24": rows, "inpt": inpt, "extras": extras}],
        core_ids=[0],
    )
    last_perf.append(("update", res_b.exec_time_ns))
    newrows = res_b.results[0]["newrows"]

    # ---- host: unshard / assemble full-shape outputs ----
    final_rsp = np.zeros(N_TOTAL, np.float32)
    final_rsp[zsel] = zscore
    final_rsp[ysel] = yscore
    neurons_out = neurons.copy()
    neurons_out[idx] = newrows
    ages_out = ages.copy()
    ages_out[idx] += np.float32(1.0)

    return final_rsp[NUM_NEURONS:], final_rsp[:NUM_NEURONS], neurons_out, ages_out


# revision 44
# speedup vs baseline: 1.0274x; 1.0214x over previous
"""Trainium2 Bass kernel for nn_DN1 (developmental-network step).

Computation (see problem reference):
  inpt = concat(x, y_response, z)                    # (13312,)
  response = neurons @ inpt                          # (9216,)  <- memory-bound bulk
  top-k selection on z part (k=8+1) and y part (k=16+1), normalized scores
  scattered into final_rsp; the 24 selected rows get an age-weighted
  running-average update + renorm.

Distribution: neurons is row-sharded across 8 NeuronCores (1152 rows each).
Launch A computes the local matvec partials per shard with a fused
multiply+accumulate (scalar_tensor_tensor with accum_out) on the Vector
engine, in natural [row, D] layout (no transpose anywhere). The tiny top-k
over 9216 values + score normalization runs on host from the gathered
response ("topk per-shard then globally reduced" degenerates to a host
reduce at this size). Launch B updates the 24 selected rows on device (PE
for the age-weighted row sums, DVE for the axpy + renorm); the host scatters
the returned rows into the full-size outputs.

Kernels are written in raw Bass (explicit engine blocks + semaphores):
the walrus build in this container only supports a single sync-wait per
instruction, which rules out the Tile scheduler's fused-wait output, so all
waits are standalone sequencer instructions.
"""
import numpy as np

import concourse.bass as bass
import concourse.mybir as mybir
from concourse.bass_utils import run_bass_kernel_spmd

F32 = mybir.dt.float32

NCORES = 8
X_SIZE, NUM_NEURONS, Z_SIZE = 4096, 8192, 1024
N_TOTAL = NUM_NEURONS + Z_SIZE          # 9216
D = X_SIZE + NUM_NEURONS + Z_SIZE       # 13312
RPC = N_TOTAL // NCORES                 # 1152 rows per core
NTILES = RPC // 128                     # 9
TOPK, ZTOPK = 16, 8
TIE_RAND = 0.5

_progs: dict = {}
last_perf: list = []  # (label, exec_time_ns or None) for the last kernel() call


def _build_matvec(nchunk=2, wbufs=4, ibc_gpsimd=False, ibc_pe=False):
    """Per-core: resp[p, t] = dot(wshard[t*128+p, :], inpt).

    nchunk: D split per row-tile (load granularity = [128, D/nchunk]);
    wbufs: load slot double-buffering depth;
    ibc_gpsimd: replicate inpt across partitions with a GpSimd
      partition_broadcast (custom op — does not compile on this walrus);
    ibc_pe: replicate inpt via ones[1,128] matmuls through PSUM + DVE
      copies, avoiding the 6.8MB stride-0 broadcast DMA. Needs the "inpt"
      input as [2, D] with row 1 = ones.
    """
    dchunk = D // nchunk
    nmm = D // 512                        # 26 psum-bank blocks
    assert not (ibc_pe and ibc_gpsimd)
    assert not ibc_pe or dchunk % 512 == 0
    # segment list (tile, lo, hi): last tile split finer so the final stt
    # (which can only start once the last DMA lands) is short
    segs = []
    for t in range(NTILES):
        if t == NTILES - 1:
            q = dchunk // 4
            for lo in range(0, D, q):
                segs.append((t, lo, lo + q))
        else:
            for c in range(nchunk):
                segs.append((t, c * dchunk, (c + 1) * dchunk))
    ntot = len(segs)
    tile_end = {}  # tile -> 1-based tick of its last stt
    for i, (t, lo, hi) in enumerate(segs):
        tile_end[t] = i + 1
    tile_cols = {}  # tile -> (first acc col, n cols)
    for i, (t, lo, hi) in enumerate(segs):
        if t not in tile_cols:
            tile_cols[t] = [i, 0]
        tile_cols[t][1] += 1
    nc = bass.Bass()
    w = nc.dram_tensor("wshard", [RPC, D], F32, kind="ExternalInput")
    inpt = nc.dram_tensor("inpt", [D + 128] if ibc_pe else [D], F32,
                          kind="ExternalInput")
    resp = nc.dram_tensor("resp", [128, NTILES], F32, kind="ExternalOutput")
    with (
        nc.sbuf_tensor([128, D], F32) as ibc,
        nc.sbuf_tensor([1, 128], F32) as ones_sb,
        nc.sbuf_tensor([128, wbufs, dchunk], F32) as wt,
        nc.sbuf_tensor([128, ntot], F32) as acc,
        nc.sbuf_tensor([128, NTILES], F32) as resp_sb,
        nc.psum_tensor([128, 8, 512], F32) as ps,
        nc.semaphore("s_gp") as s_gp,
        nc.semaphore("s_pe") as s_pe,
        nc.semaphore("s_cp") as s_cp,
        nc.semaphore("s_dve") as s_dve,
        nc.semaphore("s_red") as s_red,
        nc.semaphore("s_out") as s_out,
        nc.Block() as block,
    ):
        s_ibc = [nc.alloc_semaphore(f"s_ibc{c}") for c in range(nchunk)]
        s_wt = [nc.alloc_semaphore(f"s_w{k}") for k in range(wbufs)]

        @block.sync
        def _(sync):
            if ibc_pe:
                # stage inpt in ibc row 0 (the broadcast copies rewrite it
                # with identical values), ones tail into its own tile
                sync.dma_start(ibc[0:1, :], inpt[None, 0:D]).then_inc(
                    s_ibc[0], 16
                )
                sync.dma_start(ones_sb[:], inpt[None, D:D + 128]).then_inc(
                    s_ibc[0], 16
                )
            elif ibc_gpsimd:
                sync.dma_start(ibc[0:1, :], inpt[None, :]).then_inc(s_ibc[0], 16)
            else:
                for c in range(nchunk):
                    sync.dma_start(
                        ibc[:, c * dchunk:(c + 1) * dchunk],
                        inpt[c * dchunk:(c + 1) * dchunk].partition_broadcast(128),
                    ).then_inc(s_ibc[c], 16)
            for i, (t, lo, hi) in enumerate(segs):
                k = i % wbufs
                if i >= wbufs:
                    # slot reuse: wait until stt_{i-wbufs} consumed the slot
                    sync.wait_ge(s_dve, i - wbufs + 1)
                sync.dma_start(
                    wt[:, k, 0:hi - lo],
                    w[t * 128:(t + 1) * 128, lo:hi],
                ).then_inc(s_wt[k], 16)
            sync.wait_ge(s_red, NTILES)
            sync.dma_start(resp[:], resp_sb[:]).then_inc(s_out, 16)
            sync.wait_ge(s_out, 16)

        if ibc_gpsimd:
            @block.gpsimd
            def _(gpsimd):
                from concourse import library_config
                gpsimd.load_library(library_config.proxy)
                gpsimd.wait_ge(s_ibc[0], 16)
                nc.gpsimd.partition_broadcast(ibc[:], ibc[0:1, :]).then_inc(s_gp, 1)

        if ibc_pe:
            @block.tensor
            def _(tensor):
                tensor.wait_ge(s_ibc[0], 32)
                for m in range(nmm):
                    b = m % 8
                    if m >= 8:
                        tensor.wait_ge(s_cp, m - 7)
                    nc.tensor.matmul(
                        ps[:, b, :],
                        ones_sb[:],
                        ibc[0:1, m * 512:(m + 1) * 512],
                        start=True,
                        stop=True,
                    ).then_inc(s_pe, 1)

        @block.vector
        def _(vector):
            if ibc_pe:
                for m in range(nmm):
                    vector.wait_ge(s_pe, m + 1)
                    nc.vector.tensor_copy(
                        ibc[:, m * 512:(m + 1) * 512], ps[:, m % 8, :]
                    ).then_inc(s_cp, 1)
            cp_waited = 0
            seen_chunk = set()
            for i, (t, lo, hi) in enumerate(segs):
                k = i % wbufs
                c = lo // dchunk
                if c not in seen_chunk:
                    seen_chunk.add(c)
                    if ibc_pe:
                        need = -(-hi // 512)  # psum blocks covering this seg
                        if need > cp_waited:
                            vector.wait_ge(s_cp, need)
                            cp_waited = need
                    elif ibc_gpsimd:
                        if not seen_chunk - {c}:
                            vector.wait_ge(s_gp, 1)
                    else:
                        vector.wait_ge(s_ibc[c], 16)
                vector.wait_ge(s_wt[k], 16 * (i // wbufs + 1))
                nc.vector.scalar_tensor_tensor(
                    out=wt[:, k, 0:hi - lo],
                    in0=wt[:, k, 0:hi - lo],
                    scalar=1.0,
                    in1=ibc[:, lo:hi],
                    op0=mybir.AluOpType.mult,
                    op1=mybir.AluOpType.mult,
                    accum_out=acc[:, i:i + 1],
                ).then_inc(s_dve, 1)
                if i + 1 == tile_end[t]:
                    # fold this tile's partials into resp while later tiles
                    # are still loading (keeps the reduce off the tail)
                    col0, ncol = tile_cols[t]
                    vector.wait_ge(s_dve, tile_end[t])
                    nc.vector.reduce_sum(
                        resp_sb[:, t:t + 1],
                        acc[:, col0:col0 + ncol],
                        axis=mybir.AxisListType.X,
                    ).then_inc(s_red, 1)
    return nc


def _build_update(nk):
    """newrows[i] = normalize(s_{branch(i)} + inpt / a_i), with
    s_b = sum_j coef_j rows_j over rows j in branch b.

    Inputs are packed in one tensor to keep DMA count low:
      rowscat [nk, D + nk + 1]: cols [0:D) selected rows, [D:D+nk) the
      branch-masked coefficient matrix (lhsT layout), col D+nk = 1/ages.
    """
    mchunk = 512
    nmm = D // mchunk                    # 26
    pbufs = 8                            # PSUM banks used
    gsize = 4                            # PSUM banks consumed per DVE op
    groups = []                          # (first chunk, n chunks) per DVE group
    m0 = 0
    while m0 < nmm:
        gw = min(gsize, nmm - m0)
        assert (m0 % pbufs) + gw <= pbufs
        groups.append((m0, gw))
        m0 += gw
    ngroups = len(groups)
    # DVE tick numbers: group stts 1..ngroups, reduce, recip, tsmul
    t_reduce = ngroups + 1
    t_recip = ngroups + 2
    t_tsmul = ngroups + 3

    nc = bass.Bass()
    rowscat = nc.dram_tensor("rowscat", [nk, D + nk + 1], F32, kind="ExternalInput")
    inpt = nc.dram_tensor("inpt", [D], F32, kind="ExternalInput")
    newrows = nc.dram_tensor("newrows", [nk, D], F32, kind="ExternalOutput")
    with (
        nc.sbuf_tensor([nk, D + nk + 1], F32) as rows_sb,
        nc.sbuf_tensor([nk, D], F32) as new_sb,
        nc.sbuf_tensor([nk, D], F32) as ibc,
        nc.sbuf_tensor([nk, gsize * mchunk], F32) as sq_trash,
        nc.sbuf_tensor([nk, ngroups], F32) as sscols,
        nc.sbuf_tensor([nk, 1], F32) as ss,
        nc.sbuf_tensor([nk, 1], F32) as rn,
        nc.psum_tensor([nk, pbufs, mchunk], F32) as s_ps,
        nc.semaphore("s_row") as s_row,
        nc.semaphore("s_ibb") as s_ibb,
        nc.semaphore("s_pe") as s_pe,
        nc.semaphore("s_dve") as s_dve,
        nc.semaphore("s_act") as s_act,
        nc.semaphore("s_out") as s_out,
        nc.Block() as block,
    ):
        @block.sync
        def _(sync):
            sync.dma_start(rows_sb[:], rowscat[:]).then_inc(s_row, 16)
            sync.dma_start(ibc[:], inpt[:].partition_broadcast(nk)).then_inc(
                s_ibb, 16
            )
            sync.wait_ge(s_dve, t_tsmul)
            sync.dma_start(newrows[:], new_sb[:]).then_inc(s_out, 16)
            sync.wait_ge(s_out, 16)

        @block.tensor
        def _(tensor):
            tensor.wait_ge(s_row, 16)
            for m in range(nmm):
                b = m % pbufs
                if m >= pbufs:
                    # psum bank reuse: wait until the group that consumed
                    # chunk m-pbufs has run
                    gidx = next(
                        gi for gi, (g0, gw) in enumerate(groups)
                        if g0 <= m - pbufs < g0 + gw
                    )
                    tensor.wait_ge(s_dve, gidx + 1)
                nc.tensor.matmul(
                    s_ps[:, b, :],
                    rows_sb[:, D:D + nk],
                    rows_sb[:, m * mchunk:(m + 1) * mchunk],
                    start=True,
                    stop=True,
                ).then_inc(s_pe, 1)

        @block.vector
        def _(vector):
            vector.wait_ge(s_ibb, 16)
            vector.wait_ge(s_row, 16)
            for gi, (g0, gw) in enumerate(groups):
                b0 = g0 % pbufs
                vector.wait_ge(s_pe, g0 + gw)
                sl = slice(g0 * mchunk, (g0 + gw) * mchunk)
                nc.vector.scalar_tensor_tensor(
                    out=new_sb[:, sl],
                    in0=ibc[:, sl],
                    scalar=rows_sb[:, D + nk:D + nk + 1],
                    in1=s_ps[:, b0:b0 + gw, :],
                    op0=mybir.AluOpType.mult,
                    op1=mybir.AluOpType.add,
                ).then_inc(s_dve, 1)
            # norm tail: squares were accumulated per-group on ACT
            vector.wait_ge(s_act, ngroups)
            nc.vector.reduce_sum(
                ss[:], sscols[:], axis=mybir.AxisListType.X
            ).then_inc(s_dve, 1)
            vector.wait_ge(s_act, ngroups + 1)
            nc.vector.reciprocal(rn[:], rn[:]).then_inc(s_dve, 1)
            vector.wait_ge(s_dve, t_recip)
            nc.vector.tensor_scalar_mul(new_sb[:], new_sb[:], rn[:]).then_inc(
                s_dve, 1
            )

        @block.scalar
        def _(scalar):
            for gi, (g0, gw) in enumerate(groups):
                scalar.wait_ge(s_dve, gi + 1)
                if gi > 0:
                    # same-engine WAW on the trash buffer needs a sem
                    scalar.wait_ge(s_act, gi)
                sl = slice(g0 * mchunk, (g0 + gw) * mchunk)
                nc.scalar.activation(
                    sq_trash[:, 0:gw * mchunk],
                    new_sb[:, sl],
                    mybir.ActivationFunctionType.Square,
                    accum_out=sscols[:, gi:gi + 1],
                ).then_inc(s_act, 1)
            scalar.wait_ge(s_dve, t_reduce)
            nc.scalar.activation(
                rn[:], ss[:], mybir.ActivationFunctionType.Sqrt
            ).then_inc(s_act, 1)
    return nc


def _build_update2(nk, kz):
    """128-partition-layout row update: partition p holds segment
    [p*SEG:(p+1)*SEG) of every selected row, so every DMA and vector op runs
    at full port width (v1 ran everything on nk=24 partitions -> 3/16 SDMA
    engines and 24/128 DVE lanes).

    Inputs:
      rows24 [nk, D]    selected rows (z rows first)
      inpt   [D]
      extras [128, 128 + 2*nk]: cols [0:128) ones (matmul reducer),
             [128:128+nk) coef=(a-1)/a, [128+nk:128+2nk) 1/a
    Output:
      newrows [nk, D]

    Per-branch s = sum_i coef_i rows_i becomes a free-dim strided reduce of
    the coef-scaled rows; the cross-partition sum-of-squares reduce+broadcast
    is one ones[128,128] matmul.
    """
    seg = D // 128                       # 104
    ky = nk - kz
    nc = bass.Bass()
    rows24 = nc.dram_tensor("rows24", [nk, D], F32, kind="ExternalInput")
    inpt = nc.dram_tensor("inpt", [D], F32, kind="ExternalInput")
    extras = nc.dram_tensor("extras", [128, 128 + 2 * nk], F32, kind="ExternalInput")
    newrows = nc.dram_tensor("newrows", [nk, D], F32, kind="ExternalOutput")
    with (
        nc.sbuf_tensor([128, nk * seg], F32) as R,
        nc.sbuf_tensor([128, nk * seg], F32) as W,
        nc.sbuf_tensor([128, seg], F32) as ibc,
        nc.sbuf_tensor([128, seg], F32) as s_z,
        nc.sbuf_tensor([128, seg], F32) as s_y,
        nc.sbuf_tensor([128, 128 + 2 * nk], F32) as ex,
        nc.sbuf_tensor([128, nk], F32) as ssqp,
        nc.sbuf_tensor([128, nk], F32) as rn,
        nc.psum_tensor([128, nk], F32) as ps_ssq,
        nc.semaphore("s_r") as s_r,
        nc.semaphore("s_ry") as s_ry,
        nc.semaphore("s_i") as s_i,
        nc.semaphore("s_e") as s_e,
        nc.semaphore("s_pe") as s_pe,
        nc.semaphore("s_dve") as s_dve,
        nc.semaphore("s_act") as s_act,
        nc.semaphore("s_out") as s_out,
        nc.Block() as block,
    ):
        Rv = R[:].rearrange("p (i j) -> p i j", i=nk)      # [128, nk, seg]
        Wv = W[:].rearrange("p (i j) -> p i j", i=nk)
        coef_bc = ex[:, 128:128 + nk][:, :, None].broadcast_to((128, nk, seg))
        inva_bc = ex[:, 128 + nk:128 + 2 * nk][:, :, None].broadcast_to(
            (128, nk, seg)
        )
        ibc_bc = ibc[:][:, None, :].broadcast_to((128, nk, seg))
        rn_bc = rn[:][:, :, None].broadcast_to((128, nk, seg))

        @block.sync
        def _(sync):
            sync.dma_start(
                ibc[:], inpt[:].rearrange("(p j) -> p j", p=128)
            ).then_inc(s_i, 16)
            sync.dma_start(ex[:], extras[:]).then_inc(s_e, 16)
            # rows land per-branch so z-branch compute overlaps the y transfer
            rvi = rows24[:].rearrange("i (p j) -> p i j", p=128)
            sync.dma_start(Rv[:, 0:kz, :], rvi[:, 0:kz, :]).then_inc(s_r, 16)
            sync.dma_start(Rv[:, kz:nk, :], rvi[:, kz:nk, :]).then_inc(s_ry, 16)
            # store in row-halves so the first DMA overlaps the second scale
            nro = newrows[:].rearrange("i (p j) -> p i j", p=128)
            sync.wait_ge(s_dve, 10)
            sync.dma_start(nro[:, 0:nk // 2, :], Rv[:, 0:nk // 2, :]).then_inc(
                s_out, 16
            )
            sync.wait_ge(s_dve, 11)
            sync.dma_start(nro[:, nk // 2:, :], Rv[:, nk // 2:, :]).then_inc(
                s_out, 16
            )
            sync.wait_ge(s_out, 32)

        @block.vector
        def _(vector):
            # 1: W = inpt x (1/a) — needs only the small early inputs, so it
            # runs while the big rows DMA is still streaming
            vector.wait_ge(s_i, 16)
            vector.wait_ge(s_e, 16)
            nc.vector.tensor_mul(Wv, ibc_bc, inva_bc).then_inc(s_dve, 1)
            # 2-4: z branch (runs while the y rows are still landing)
            vector.wait_ge(s_r, 16)
            nc.vector.tensor_mul(
                Rv[:, 0:kz, :], Rv[:, 0:kz, :], coef_bc[:, 0:kz, :]
            ).then_inc(s_dve, 1)
            vector.wait_ge(s_dve, 2)
            nc.vector.reduce_sum(
                s_z[:], Rv[:, 0:kz, :].transpose([0, 2, 1]),
                axis=mybir.AxisListType.X,
            ).then_inc(s_dve, 1)
            vector.wait_ge(s_dve, 3)
            nc.vector.tensor_add(
                Rv[:, 0:kz, :], Wv[:, 0:kz, :],
                s_z[:][:, None, :].broadcast_to((128, kz, seg)),
            ).then_inc(s_dve, 1)
            # 5-7: y branch
            vector.wait_ge(s_ry, 16)
            nc.vector.tensor_mul(
                Rv[:, kz:nk, :], Rv[:, kz:nk, :], coef_bc[:, kz:nk, :]
            ).then_inc(s_dve, 1)
            vector.wait_ge(s_dve, 5)
            nc.vector.reduce_sum(
                s_y[:], Rv[:, kz:nk, :].transpose([0, 2, 1]),
                axis=mybir.AxisListType.X,
            ).then_inc(s_dve, 1)
            vector.wait_ge(s_dve, 6)
            nc.vector.tensor_add(
                Rv[:, kz:nk, :], Wv[:, kz:nk, :],
                s_y[:][:, None, :].broadcast_to((128, ky, seg)),
            ).then_inc(s_dve, 1)
            # 8: per-partition sumsq segments (squares by ACT into W)
            vector.wait_ge(s_act, 1)
            nc.vector.reduce_sum(
                ssqp[:], Wv, axis=mybir.AxisListType.X
            ).then_inc(s_dve, 1)
            # 9: 1/sqrt after PE reduce-broadcast + ACT sqrt
            vector.wait_ge(s_act, 2)
            nc.vector.reciprocal(rn[:], rn[:]).then_inc(s_dve, 1)
            vector.wait_ge(s_dve, 9)
            # 10,11: scale rows by 1/norm, in halves (overlaps the first store)
            h = nk // 2
            nc.vector.tensor_mul(
                Rv[:, 0:h, :], Rv[:, 0:h, :],
                rn[:, 0:h][:, :, None].broadcast_to((128, h, seg)),
            ).then_inc(s_dve, 1)
            nc.vector.tensor_mul(
                Rv[:, h:nk, :], Rv[:, h:nk, :],
                rn[:, h:nk][:, :, None].broadcast_to((128, nk - h, seg)),
            ).then_inc(s_dve, 1)

        @block.tensor
        def _(tensor):
            tensor.wait_ge(s_dve, 8)
            # ones[128,128] @ ssqp: cross-partition sum AND broadcast in one op
            nc.tensor.matmul(
                ps_ssq[:], ex[:, 0:128], ssqp[:], start=True, stop=True
            ).then_inc(s_pe, 1)

        @block.scalar
        def _(scalar):
            scalar.wait_ge(s_dve, 7)
            nc.scalar.activation(
                Wv, Rv, mybir.ActivationFunctionType.Square
            ).then_inc(s_act, 1)
            scalar.wait_ge(s_pe, 1)
            nc.scalar.activation(
                rn[:], ps_ssq[:], mybir.ActivationFunctionType.Sqrt
            ).then_inc(s_act, 1)
    return nc


def _get_prog(key, builder):
    if key not in _progs:
        _progs[key] = builder()
    return _progs[key]


def kernel(x, z, neurons, ages, y_response, num_neurons_init):
    global last_perf
    last_perf = []
    x = np.ascontiguousarray(np.asarray(x, dtype=np.float32))
    z = np.ascontiguousarray(np.asarray(z, dtype=np.float32))
    neurons = np.ascontiguousarray(np.asarray(neurons, dtype=np.float32))
    ages = np.ascontiguousarray(np.asarray(ages, dtype=np.float32))
    y_response = np.ascontiguousarray(np.asarray(y_response, dtype=np.float32))
    nni = int(np.asarray(num_neurons_init))

    inpt = np.concatenate([x, y_response, z]).astype(np.float32)
    inpt_ones = np.concatenate([inpt, np.ones(128, np.float32)])

    # ---- launch A: distributed matvec (row-sharded) ----
    nc_a = _get_prog(
        "matvec", lambda: _build_matvec(nchunk=2, wbufs=5, ibc_pe=True)
    )
    in_maps = [
        {"wshard": neurons[c * RPC:(c + 1) * RPC], "inpt": inpt_ones}
        for c in range(NCORES)
    ]
    res_a = run_bass_kernel_spmd(nc_a, in_maps, core_ids=list(range(NCORES)))
    last_perf.append(("matvec", res_a.exec_time_ns))
    response = np.concatenate(
        [res_a.results[c]["resp"].T.ravel() for c in range(NCORES)]
    )

    # ---- host: global top-k reduce + normalized scores (tiny: 9216 values) ----
    ytk = 1 if nni <= TOPK else TOPK
    ztk = 1 if nni <= TOPK else ZTOPK

    zresp = response[NUM_NEURONS:]
    zord = np.argsort(-zresp, kind="stable")[:ztk + 1]
    zvals = zresp[zord]
    zsel = NUM_NEURONS + zord[:-1]
    zscore = (zvals[:-1] - zvals[-1]) / (zvals[0] - zvals[-1])

    yresp = response[:NUM_NEURONS]
    yord = np.argsort(-yresp, kind="stable")[:ytk + 1]
    yvals = yresp[yord]
    t = np.float32(1.0 if np.any(yvals[:-1] == yvals[-1]) else 0.0)
    denom = yvals[0] - yvals[-1] + np.float32(1e-9) * (t * np.float32(TIE_RAND))
    ysel = yord[:-1]
    yscore = (yvals[:-1] - yvals[-1]) / denom

    # ---- launch B: update + renorm the selected rows on device ----
    idx = np.concatenate([zsel, ysel])
    nk = len(idx)
    a_sel = ages[idx]
    coef = ((a_sel - np.float32(1.0)) / a_sel).astype(np.float32)
    inva = (np.float32(1.0) / a_sel).astype(np.float32)
    extras = np.ones((128, 128 + 2 * nk), np.float32)
    extras[:, 128:128 + nk] = coef[None, :]
    extras[:, 128 + nk:] = inva[None, :]
    rows = np.ascontiguousarray(neurons[idx])

    nc_b = _get_prog(("update2", nk, ztk), lambda: _build_update2(nk, ztk))
    res_b = run_bass_kernel_spmd(
        nc_b,
        [{"rows24": rows, "inpt": inpt, "extras": extras}],
        core_ids=[0],
    )
    last_perf.append(("update", res_b.exec_time_ns))
    newrows = res_b.results[0]["newrows"]

    # ---- host: unshard / assemble full-shape outputs ----
    final_rsp = np.zeros(N_TOTAL, np.float32)
    final_rsp[zsel] = zscore
    final_rsp[ysel] = yscore
    neurons_out = neurons.copy()
    neurons_out[idx] = newrows
    ages_out = ages.copy()
    ages_out[idx] += np.float32(1.0)

    return final_rsp[NUM_NEURONS:], final_rsp[:NUM_NEURONS], neurons_out, ages_out


# revision 45
# speedup vs baseline: 1.0529x; 1.0248x over previous
"""Trainium2 Bass kernel for nn_DN1 (developmental-network step).

Computation (see problem reference):
  inpt = concat(x, y_response, z)                    # (13312,)
  response = neurons @ inpt                          # (9216,)  <- memory-bound bulk
  top-k selection on z part (k=8+1) and y part (k=16+1), normalized scores
  scattered into final_rsp; the 24 selected rows get an age-weighted
  running-average update + renorm.

Distribution: neurons is row-sharded across 8 NeuronCores (1152 rows each).
Launch A computes the local matvec partials per shard with a fused
multiply+accumulate (scalar_tensor_tensor with accum_out) on the Vector
engine, in natural [row, D] layout (no transpose anywhere). The tiny top-k
over 9216 values + score normalization runs on host from the gathered
response ("topk per-shard then globally reduced" degenerates to a host
reduce at this size). Launch B updates the 24 selected rows on device (PE
for the age-weighted row sums, DVE for the axpy + renorm); the host scatters
the returned rows into the full-size outputs.

Kernels are written in raw Bass (explicit engine blocks + semaphores):
the walrus build in this container only supports a single sync-wait per
instruction, which rules out the Tile scheduler's fused-wait output, so all
waits are standalone sequencer instructions.
"""
import numpy as np

import concourse.bass as bass
import concourse.mybir as mybir
from concourse.bass_utils import run_bass_kernel_spmd

F32 = mybir.dt.float32

NCORES = 8
X_SIZE, NUM_NEURONS, Z_SIZE = 4096, 8192, 1024
N_TOTAL = NUM_NEURONS + Z_SIZE          # 9216
D = X_SIZE + NUM_NEURONS + Z_SIZE       # 13312
RPC = N_TOTAL // NCORES                 # 1152 rows per core
NTILES = RPC // 128                     # 9
TOPK, ZTOPK = 16, 8
TIE_RAND = 0.5

_progs: dict = {}
last_perf: list = []  # (label, exec_time_ns or None) for the last kernel() call


def _build_matvec(nchunk=2, wbufs=4, ibc_gpsimd=False, ibc_pe=False):
    """Per-core: resp[p, t] = dot(wshard[t*128+p, :], inpt).

    nchunk: D split per row-tile (load granularity = [128, D/nchunk]);
    wbufs: load slot double-buffering depth;
    ibc_gpsimd: replicate inpt across partitions with a GpSimd
      partition_broadcast (custom op — does not compile on this walrus);
    ibc_pe: replicate inpt via ones[1,128] matmuls through PSUM + DVE
      copies, avoiding the 6.8MB stride-0 broadcast DMA. Needs the "inpt"
      input as [2, D] with row 1 = ones.
    """
    dchunk = D // nchunk
    nmm = D // 512                        # 26 psum-bank blocks
    assert not (ibc_pe and ibc_gpsimd)
    assert not ibc_pe or dchunk % 512 == 0
    # segment list (tile, lo, hi): last tile split finer so the final stt
    # (which can only start once the last DMA lands) is short
    segs = []
    for t in range(NTILES):
        if t == NTILES - 1:
            q = dchunk // 4
            for lo in range(0, D, q):
                segs.append((t, lo, lo + q))
        else:
            for c in range(nchunk):
                segs.append((t, c * dchunk, (c + 1) * dchunk))
    ntot = len(segs)
    tile_end = {}  # tile -> 1-based tick of its last stt
    for i, (t, lo, hi) in enumerate(segs):
        tile_end[t] = i + 1
    tile_cols = {}  # tile -> (first acc col, n cols)
    for i, (t, lo, hi) in enumerate(segs):
        if t not in tile_cols:
            tile_cols[t] = [i, 0]
        tile_cols[t][1] += 1
    nc = bass.Bass()
    w = nc.dram_tensor("wshard", [RPC, D], F32, kind="ExternalInput")
    inpt = nc.dram_tensor("inpt", [D + 128] if ibc_pe else [D], F32,
                          kind="ExternalInput")
    resp = nc.dram_tensor("resp", [128, NTILES], F32, kind="ExternalOutput")
    with (
        nc.sbuf_tensor([128, D], F32) as ibc,
        nc.sbuf_tensor([1, 128], F32) as ones_sb,
        nc.sbuf_tensor([128, wbufs, dchunk], F32) as wt,
        nc.sbuf_tensor([128, ntot], F32) as acc,
        nc.sbuf_tensor([128, NTILES], F32) as resp_sb,
        nc.psum_tensor([128, 8, 512], F32) as ps,
        nc.semaphore("s_gp") as s_gp,
        nc.semaphore("s_pe") as s_pe,
        nc.semaphore("s_cp") as s_cp,
        nc.semaphore("s_dve") as s_dve,
        nc.semaphore("s_red") as s_red,
        nc.semaphore("s_out") as s_out,
        nc.Block() as block,
    ):
        s_ibc = [nc.alloc_semaphore(f"s_ibc{c}") for c in range(nchunk)]
        s_wt = [nc.alloc_semaphore(f"s_w{k}") for k in range(wbufs)]

        @block.sync
        def _(sync):
            if ibc_pe:
                # stage inpt in ibc row 0 (the broadcast copies rewrite it
                # with identical values), ones tail into its own tile
                sync.dma_start(ibc[0:1, :], inpt[None, 0:D]).then_inc(
                    s_ibc[0], 16
                )
                sync.dma_start(ones_sb[:], inpt[None, D:D + 128]).then_inc(
                    s_ibc[0], 16
                )
            elif ibc_gpsimd:
                sync.dma_start(ibc[0:1, :], inpt[None, :]).then_inc(s_ibc[0], 16)
            else:
                for c in range(nchunk):
                    sync.dma_start(
                        ibc[:, c * dchunk:(c + 1) * dchunk],
                        inpt[c * dchunk:(c + 1) * dchunk].partition_broadcast(128),
                    ).then_inc(s_ibc[c], 16)
            for i, (t, lo, hi) in enumerate(segs):
                k = i % wbufs
                if i >= wbufs:
                    # slot reuse: wait until stt_{i-wbufs} consumed the slot
                    sync.wait_ge(s_dve, i - wbufs + 1)
                sync.dma_start(
                    wt[:, k, 0:hi - lo],
                    w[t * 128:(t + 1) * 128, lo:hi],
                ).then_inc(s_wt[k], 16)
            sync.wait_ge(s_red, NTILES)
            sync.dma_start(resp[:], resp_sb[:]).then_inc(s_out, 16)
            sync.wait_ge(s_out, 16)

        if ibc_gpsimd:
            @block.gpsimd
            def _(gpsimd):
                from concourse import library_config
                gpsimd.load_library(library_config.proxy)
                gpsimd.wait_ge(s_ibc[0], 16)
                nc.gpsimd.partition_broadcast(ibc[:], ibc[0:1, :]).then_inc(s_gp, 1)

        if ibc_pe:
            @block.tensor
            def _(tensor):
                tensor.wait_ge(s_ibc[0], 32)
                for m in range(nmm):
                    b = m % 8
                    if m >= 8:
                        tensor.wait_ge(s_cp, m - 7)
                    nc.tensor.matmul(
                        ps[:, b, :],
                        ones_sb[:],
                        ibc[0:1, m * 512:(m + 1) * 512],
                        start=True,
                        stop=True,
                    ).then_inc(s_pe, 1)

        @block.vector
        def _(vector):
            if ibc_pe:
                for m in range(nmm):
                    vector.wait_ge(s_pe, m + 1)
                    nc.vector.tensor_copy(
                        ibc[:, m * 512:(m + 1) * 512], ps[:, m % 8, :]
                    ).then_inc(s_cp, 1)
            cp_waited = 0
            seen_chunk = set()
            for i, (t, lo, hi) in enumerate(segs):
                k = i % wbufs
                c = lo // dchunk
                if c not in seen_chunk:
                    seen_chunk.add(c)
                    if ibc_pe:
                        need = -(-hi // 512)  # psum blocks covering this seg
                        if need > cp_waited:
                            vector.wait_ge(s_cp, need)
                            cp_waited = need
                    elif ibc_gpsimd:
                        if not seen_chunk - {c}:
                            vector.wait_ge(s_gp, 1)
                    else:
                        vector.wait_ge(s_ibc[c], 16)
                vector.wait_ge(s_wt[k], 16 * (i // wbufs + 1))
                nc.vector.scalar_tensor_tensor(
                    out=wt[:, k, 0:hi - lo],
                    in0=wt[:, k, 0:hi - lo],
                    scalar=1.0,
                    in1=ibc[:, lo:hi],
                    op0=mybir.AluOpType.mult,
                    op1=mybir.AluOpType.mult,
                    accum_out=acc[:, i:i + 1],
                ).then_inc(s_dve, 1)
                if i + 1 == tile_end[t]:
                    # fold this tile's partials into resp while later tiles
                    # are still loading (keeps the reduce off the tail)
                    col0, ncol = tile_cols[t]
                    vector.wait_ge(s_dve, tile_end[t])
                    nc.vector.reduce_sum(
                        resp_sb[:, t:t + 1],
                        acc[:, col0:col0 + ncol],
                        axis=mybir.AxisListType.X,
                    ).then_inc(s_red, 1)
    return nc


def _build_update(nk):
    """newrows[i] = normalize(s_{branch(i)} + inpt / a_i), with
    s_b = sum_j coef_j rows_j over rows j in branch b.

    Inputs are packed in one tensor to keep DMA count low:
      rowscat [nk, D + nk + 1]: cols [0:D) selected rows, [D:D+nk) the
      branch-masked coefficient matrix (lhsT layout), col D+nk = 1/ages.
    """
    mchunk = 512
    nmm = D // mchunk                    # 26
    pbufs = 8                            # PSUM banks used
    gsize = 4                            # PSUM banks consumed per DVE op
    groups = []                          # (first chunk, n chunks) per DVE group
    m0 = 0
    while m0 < nmm:
        gw = min(gsize, nmm - m0)
        assert (m0 % pbufs) + gw <= pbufs
        groups.append((m0, gw))
        m0 += gw
    ngroups = len(groups)
    # DVE tick numbers: group stts 1..ngroups, reduce, recip, tsmul
    t_reduce = ngroups + 1
    t_recip = ngroups + 2
    t_tsmul = ngroups + 3

    nc = bass.Bass()
    rowscat = nc.dram_tensor("rowscat", [nk, D + nk + 1], F32, kind="ExternalInput")
    inpt = nc.dram_tensor("inpt", [D], F32, kind="ExternalInput")
    newrows = nc.dram_tensor("newrows", [128, nk * seg], F32, kind="ExternalOutput")
    with (
        nc.sbuf_tensor([nk, D + nk + 1], F32) as rows_sb,
        nc.sbuf_tensor([nk, D], F32) as new_sb,
        nc.sbuf_tensor([nk, D], F32) as ibc,
        nc.sbuf_tensor([nk, gsize * mchunk], F32) as sq_trash,
        nc.sbuf_tensor([nk, ngroups], F32) as sscols,
        nc.sbuf_tensor([nk, 1], F32) as ss,
        nc.sbuf_tensor([nk, 1], F32) as rn,
        nc.psum_tensor([nk, pbufs, mchunk], F32) as s_ps,
        nc.semaphore("s_row") as s_row,
        nc.semaphore("s_ibb") as s_ibb,
        nc.semaphore("s_pe") as s_pe,
        nc.semaphore("s_dve") as s_dve,
        nc.semaphore("s_act") as s_act,
        nc.semaphore("s_out") as s_out,
        nc.Block() as block,
    ):
        @block.sync
        def _(sync):
            sync.dma_start(rows_sb[:], rowscat[:]).then_inc(s_row, 16)
            sync.dma_start(ibc[:], inpt[:].partition_broadcast(nk)).then_inc(
                s_ibb, 16
            )
            sync.wait_ge(s_dve, t_tsmul)
            sync.dma_start(newrows[:], new_sb[:]).then_inc(s_out, 16)
            sync.wait_ge(s_out, 16)

        @block.tensor
        def _(tensor):
            tensor.wait_ge(s_row, 16)
            for m in range(nmm):
                b = m % pbufs
                if m >= pbufs:
                    # psum bank reuse: wait until the group that consumed
                    # chunk m-pbufs has run
                    gidx = next(
                        gi for gi, (g0, gw) in enumerate(groups)
                        if g0 <= m - pbufs < g0 + gw
                    )
                    tensor.wait_ge(s_dve, gidx + 1)
                nc.tensor.matmul(
                    s_ps[:, b, :],
                    rows_sb[:, D:D + nk],
                    rows_sb[:, m * mchunk:(m + 1) * mchunk],
                    start=True,
                    stop=True,
                ).then_inc(s_pe, 1)

        @block.vector
        def _(vector):
            vector.wait_ge(s_ibb, 16)
            vector.wait_ge(s_row, 16)
            for gi, (g0, gw) in enumerate(groups):
                b0 = g0 % pbufs
                vector.wait_ge(s_pe, g0 + gw)
                sl = slice(g0 * mchunk, (g0 + gw) * mchunk)
                nc.vector.scalar_tensor_tensor(
                    out=new_sb[:, sl],
                    in0=ibc[:, sl],
                    scalar=rows_sb[:, D + nk:D + nk + 1],
                    in1=s_ps[:, b0:b0 + gw, :],
                    op0=mybir.AluOpType.mult,
                    op1=mybir.AluOpType.add,
                ).then_inc(s_dve, 1)
            # norm tail: squares were accumulated per-group on ACT
            vector.wait_ge(s_act, ngroups)
            nc.vector.reduce_sum(
                ss[:], sscols[:], axis=mybir.AxisListType.X
            ).then_inc(s_dve, 1)
            vector.wait_ge(s_act, ngroups + 1)
            nc.vector.reciprocal(rn[:], rn[:]).then_inc(s_dve, 1)
            vector.wait_ge(s_dve, t_recip)
            nc.vector.tensor_scalar_mul(new_sb[:], new_sb[:], rn[:]).then_inc(
                s_dve, 1
            )

        @block.scalar
        def _(scalar):
            for gi, (g0, gw) in enumerate(groups):
                scalar.wait_ge(s_dve, gi + 1)
                if gi > 0:
                    # same-engine WAW on the trash buffer needs a sem
                    scalar.wait_ge(s_act, gi)
                sl = slice(g0 * mchunk, (g0 + gw) * mchunk)
                nc.scalar.activation(
                    sq_trash[:, 0:gw * mchunk],
                    new_sb[:, sl],
                    mybir.ActivationFunctionType.Square,
                    accum_out=sscols[:, gi:gi + 1],
                ).then_inc(s_act, 1)
            scalar.wait_ge(s_dve, t_reduce)
            nc.scalar.activation(
                rn[:], ss[:], mybir.ActivationFunctionType.Sqrt
            ).then_inc(s_act, 1)
    return nc


def _build_update2(nk, kz):
    """128-partition-layout row update: partition p holds segment
    [p*SEG:(p+1)*SEG) of every selected row, so every DMA and vector op runs
    at full port width (v1 ran everything on nk=24 partitions -> 3/16 SDMA
    engines and 24/128 DVE lanes).

    Inputs:
      rows24 [nk, D]    selected rows (z rows first)
      inpt   [D]
      extras [128, 128 + 2*nk]: cols [0:128) ones (matmul reducer),
             [128:128+nk) coef=(a-1)/a, [128+nk:128+2nk) 1/a
    Output:
      newrows [nk, D]

    Per-branch s = sum_i coef_i rows_i becomes a free-dim strided reduce of
    the coef-scaled rows; the cross-partition sum-of-squares reduce+broadcast
    is one ones[128,128] matmul.
    """
    seg = D // 128                       # 104
    ky = nk - kz
    nc = bass.Bass()
    rows24 = nc.dram_tensor("rows24", [128, nk * seg], F32, kind="ExternalInput")
    inpt = nc.dram_tensor("inpt", [D], F32, kind="ExternalInput")
    extras = nc.dram_tensor("extras", [128, 128 + 2 * nk], F32, kind="ExternalInput")
    newrows = nc.dram_tensor("newrows", [128, nk * seg], F32, kind="ExternalOutput")
    with (
        nc.sbuf_tensor([128, nk * seg], F32) as R,
        nc.sbuf_tensor([128, nk * seg], F32) as W,
        nc.sbuf_tensor([128, seg], F32) as ibc,
        nc.sbuf_tensor([128, seg], F32) as s_z,
        nc.sbuf_tensor([128, seg], F32) as s_y,
        nc.sbuf_tensor([128, 128 + 2 * nk], F32) as ex,
        nc.sbuf_tensor([128, nk], F32) as ssqp,
        nc.sbuf_tensor([128, nk], F32) as rn,
        nc.psum_tensor([128, nk], F32) as ps_ssq,
        nc.semaphore("s_r") as s_r,
        nc.semaphore("s_ry") as s_ry,
        nc.semaphore("s_i") as s_i,
        nc.semaphore("s_e") as s_e,
        nc.semaphore("s_pe") as s_pe,
        nc.semaphore("s_dve") as s_dve,
        nc.semaphore("s_act") as s_act,
        nc.semaphore("s_out") as s_out,
        nc.Block() as block,
    ):
        Rv = R[:].rearrange("p (i j) -> p i j", i=nk)      # [128, nk, seg]
        Wv = W[:].rearrange("p (i j) -> p i j", i=nk)
        coef_bc = ex[:, 128:128 + nk][:, :, None].broadcast_to((128, nk, seg))
        inva_bc = ex[:, 128 + nk:128 + 2 * nk][:, :, None].broadcast_to(
            (128, nk, seg)
        )
        ibc_bc = ibc[:][:, None, :].broadcast_to((128, nk, seg))
        rn_bc = rn[:][:, :, None].broadcast_to((128, nk, seg))

        @block.sync
        def _(sync):
            sync.dma_start(
                ibc[:], inpt[:].rearrange("(p j) -> p j", p=128)
            ).then_inc(s_i, 16)
            sync.dma_start(ex[:], extras[:]).then_inc(s_e, 16)
            # rows land per-branch so z-branch compute overlaps the y transfer
            # (host supplies device layout, so both transfers are contiguous)
            sync.dma_start(
                R[:, 0:kz * seg], rows24[:, 0:kz * seg]
            ).then_inc(s_r, 16)
            sync.dma_start(
                R[:, kz * seg:], rows24[:, kz * seg:]
            ).then_inc(s_ry, 16)
            # store in row-halves so the first DMA overlaps the second scale
            hs = (nk // 2) * seg
            sync.wait_ge(s_dve, 10)
            sync.dma_start(newrows[:, 0:hs], R[:, 0:hs]).then_inc(s_out, 16)
            sync.wait_ge(s_dve, 11)
            sync.dma_start(newrows[:, hs:], R[:, hs:]).then_inc(s_out, 16)
            sync.wait_ge(s_out, 32)

        @block.vector
        def _(vector):
            # 1: W = inpt x (1/a) — needs only the small early inputs, so it
            # runs while the big rows DMA is still streaming
            vector.wait_ge(s_i, 16)
            vector.wait_ge(s_e, 16)
            nc.vector.tensor_mul(Wv, ibc_bc, inva_bc).then_inc(s_dve, 1)
            # 2-4: z branch (runs while the y rows are still landing)
            vector.wait_ge(s_r, 16)
            nc.vector.tensor_mul(
                Rv[:, 0:kz, :], Rv[:, 0:kz, :], coef_bc[:, 0:kz, :]
            ).then_inc(s_dve, 1)
            vector.wait_ge(s_dve, 2)
            nc.vector.reduce_sum(
                s_z[:], Rv[:, 0:kz, :].transpose([0, 2, 1]),
                axis=mybir.AxisListType.X,
            ).then_inc(s_dve, 1)
            vector.wait_ge(s_dve, 3)
            nc.vector.tensor_add(
                Rv[:, 0:kz, :], Wv[:, 0:kz, :],
                s_z[:][:, None, :].broadcast_to((128, kz, seg)),
            ).then_inc(s_dve, 1)
            # 5-7: y branch
            vector.wait_ge(s_ry, 16)
            nc.vector.tensor_mul(
                Rv[:, kz:nk, :], Rv[:, kz:nk, :], coef_bc[:, kz:nk, :]
            ).then_inc(s_dve, 1)
            vector.wait_ge(s_dve, 5)
            nc.vector.reduce_sum(
                s_y[:], Rv[:, kz:nk, :].transpose([0, 2, 1]),
                axis=mybir.AxisListType.X,
            ).then_inc(s_dve, 1)
            vector.wait_ge(s_dve, 6)
            nc.vector.tensor_add(
                Rv[:, kz:nk, :], Wv[:, kz:nk, :],
                s_y[:][:, None, :].broadcast_to((128, ky, seg)),
            ).then_inc(s_dve, 1)
            # 8: per-partition sumsq segments (squares by ACT into W)
            vector.wait_ge(s_act, 1)
            nc.vector.reduce_sum(
                ssqp[:], Wv, axis=mybir.AxisListType.X
            ).then_inc(s_dve, 1)
            # 9: 1/sqrt after PE reduce-broadcast + ACT sqrt
            vector.wait_ge(s_act, 2)
            nc.vector.reciprocal(rn[:], rn[:]).then_inc(s_dve, 1)
            vector.wait_ge(s_dve, 9)
            # 10,11: scale rows by 1/norm, in halves (overlaps the first store)
            h = nk // 2
            nc.vector.tensor_mul(
                Rv[:, 0:h, :], Rv[:, 0:h, :],
                rn[:, 0:h][:, :, None].broadcast_to((128, h, seg)),
            ).then_inc(s_dve, 1)
            nc.vector.tensor_mul(
                Rv[:, h:nk, :], Rv[:, h:nk, :],
                rn[:, h:nk][:, :, None].broadcast_to((128, nk - h, seg)),
            ).then_inc(s_dve, 1)

        @block.tensor
        def _(tensor):
            tensor.wait_ge(s_dve, 8)
            # ones[128,128] @ ssqp: cross-partition sum AND broadcast in one op
            nc.tensor.matmul(
                ps_ssq[:], ex[:, 0:128], ssqp[:], start=True, stop=True
            ).then_inc(s_pe, 1)

        @block.scalar
        def _(scalar):
            scalar.wait_ge(s_dve, 7)
            nc.scalar.activation(
                Wv, Rv, mybir.ActivationFunctionType.Square
            ).then_inc(s_act, 1)
            scalar.wait_ge(s_pe, 1)
            nc.scalar.activation(
                rn[:], ps_ssq[:], mybir.ActivationFunctionType.Sqrt
            ).then_inc(s_act, 1)
    return nc


def _get_prog(key, builder):
    if key not in _progs:
        _progs[key] = builder()
    return _progs[key]


def kernel(x, z, neurons, ages, y_response, num_neurons_init):
    global last_perf
    last_perf = []
    x = np.ascontiguousarray(np.asarray(x, dtype=np.float32))
    z = np.ascontiguousarray(np.asarray(z, dtype=np.float32))
    neurons = np.ascontiguousarray(np.asarray(neurons, dtype=np.float32))
    ages = np.ascontiguousarray(np.asarray(ages, dtype=np.float32))
    y_response = np.ascontiguousarray(np.asarray(y_response, dtype=np.float32))
    nni = int(np.asarray(num_neurons_init))

    inpt = np.concatenate([x, y_response, z]).astype(np.float32)
    inpt_ones = np.concatenate([inpt, np.ones(128, np.float32)])

    # ---- launch A: distributed matvec (row-sharded) ----
    nc_a = _get_prog(
        "matvec", lambda: _build_matvec(nchunk=2, wbufs=5, ibc_pe=True)
    )
    in_maps = [
        {"wshard": neurons[c * RPC:(c + 1) * RPC], "inpt": inpt_ones}
        for c in range(NCORES)
    ]
    res_a = run_bass_kernel_spmd(nc_a, in_maps, core_ids=list(range(NCORES)))
    last_perf.append(("matvec", res_a.exec_time_ns))
    response = np.concatenate(
        [res_a.results[c]["resp"].T.ravel() for c in range(NCORES)]
    )

    # ---- host: global top-k reduce + normalized scores (tiny: 9216 values) ----
    ytk = 1 if nni <= TOPK else TOPK
    ztk = 1 if nni <= TOPK else ZTOPK

    zresp = response[NUM_NEURONS:]
    zord = np.argsort(-zresp, kind="stable")[:ztk + 1]
    zvals = zresp[zord]
    zsel = NUM_NEURONS + zord[:-1]
    zscore = (zvals[:-1] - zvals[-1]) / (zvals[0] - zvals[-1])

    yresp = response[:NUM_NEURONS]
    yord = np.argsort(-yresp, kind="stable")[:ytk + 1]
    yvals = yresp[yord]
    t = np.float32(1.0 if np.any(yvals[:-1] == yvals[-1]) else 0.0)
    denom = yvals[0] - yvals[-1] + np.float32(1e-9) * (t * np.float32(TIE_RAND))
    ysel = yord[:-1]
    yscore = (yvals[:-1] - yvals[-1]) / denom

    # ---- launch B: update + renorm the selected rows on device ----
    idx = np.concatenate([zsel, ysel])
    nk = len(idx)
    a_sel = ages[idx]
    coef = ((a_sel - np.float32(1.0)) / a_sel).astype(np.float32)
    inva = (np.float32(1.0) / a_sel).astype(np.float32)
    extras = np.ones((128, 128 + 2 * nk), np.float32)
    extras[:, 128:128 + nk] = coef[None, :]
    extras[:, 128 + nk:] = inva[None, :]
    seg = D // 128
    rows = np.ascontiguousarray(
        neurons[idx].reshape(nk, 128, seg).transpose(1, 0, 2).reshape(128, -1)
    )

    nc_b = _get_prog(("update2", nk, ztk), lambda: _build_update2(nk, ztk))
    res_b = run_bass_kernel_spmd(
        nc_b,
        [{"rows24": rows, "inpt": inpt, "extras": extras}],
        core_ids=[0],
    )
    last_perf.append(("update", res_b.exec_time_ns))
    newrows = (
        res_b.results[0]["newrows"]
        .reshape(128, nk, seg).transpose(1, 0, 2).reshape(nk, D)
    )

    # ---- host: unshard / assemble full-shape outputs ----
    final_rsp = np.zeros(N_TOTAL, np.float32)
    final_rsp[zsel] = zscore
    final_rsp[ysel] = yscore
    neurons_out = neurons.copy()
    neurons_out[idx] = newrows
    ages_out = ages.copy()
    ages_out[idx] += np.float32(1.0)

    return final_rsp[NUM_NEURONS:], final_rsp[:NUM_NEURONS], neurons_out, ages_out
